# revision 1
# baseline (speedup 1.0000x reference)
"""Trainium2 Bass kernel for per-position grouped-query attention.

Reference computation (B=4, S=4096, HID=2048, H=16, G=4, D=128, KV=512):
    q = x @ Wq + bq ; k = x @ Wk + bk ; v = x @ Wv + bv
    scores[t,h,g] = <q[t,h,:], k[t,g,:]> / sqrt(D)     (same-position only)
    probs = softmax_g(scores)
    o[t,h,:] = sum_g probs[t,h,g] * v[t,g,:]
    y = o @ Wo + bo

Strategy: data-parallel over the 16384 flattened tokens -> 2048 tokens/core
on 8 cores, all weights replicated, no collectives.  Per core the kernel is
PE-bound (~43 GFLOP bf16 -> 546us matmul floor @78.6TF/s).  The fast path:
  - x arrives host-pre-transposed per token tile (no PE transposes on the
    input side); QKV matmuls use x-tile blocks as the stationary operand.
  - attention middle: fused mult+accum score dots + weighted-sum chains on
    DVE, PSUM->SBUF casts + exp on ACT - all hidden under the PE stream.
  - O^T comes from the DMA xbar (dma_start_transpose), so the PE runs
    matmuls only; Wo matmuls consume each tile 3 pipeline steps later and
    y goes out via ACT copy + DMA.
  - weight DMAs are emitted in tile-0's consumption order (two halves of
    three interleaved psum group chains), so the DMA-bound startup overlaps
    the first two tiles' compute.
"""

import os
import sys

import numpy as np

sys.path.insert(0, "/opt/trn_rl_repo")

import ml_dtypes  # noqa: E402
from contextlib import ExitStack  # noqa: E402

import concourse.bass as bass  # noqa: E402
import concourse.bacc as bacc  # noqa: E402
import concourse.mybir as mybir  # noqa: E402
import concourse.tile as tile  # noqa: E402
from concourse.bass import ds  # noqa: E402
from concourse.bass_utils import run_bass_kernel_spmd  # noqa: E402
from concourse.masks import make_identity  # noqa: E402

B, S, HID = 4, 4096, 2048
H, G = 16, 4
D = HID // H          # 128
KV = HID * G // H     # 512
NCORES = 8
NTOK = B * S          # 16384
TPC = NTOK // NCORES  # 2048 tokens per core
P = 128
NTT = TPC // P        # 16 token tiles per core
NI = HID // P         # 16 input-feature blocks
SCALE = 1.0 / float(np.sqrt(D))

BF16 = mybir.dt.bfloat16
F32 = mybir.dt.float32

_cache = {}


def _build_fast() -> bass.Bass:
    """No-bias fast path."""
    nc = bacc.Bacc("TRN2")
    # xt: host-pretransposed per token tile: row (t*128+p), col (i*128+tok)
    # holds x[t*128+tok, i*128+p]  -> per tile a plain [128, 2048] slice whose
    # block i is the lhsT [feat-in-block, token] for the QKV matmuls.
    xt = nc.dram_tensor("xt", [TPC, HID], BF16, kind="ExternalInput")
    wq = nc.dram_tensor("wq", [HID, HID], BF16, kind="ExternalInput")
    wk = nc.dram_tensor("wk", [HID, KV], BF16, kind="ExternalInput")
    wv = nc.dram_tensor("wv", [HID, KV], BF16, kind="ExternalInput")
    wo = nc.dram_tensor("wo", [HID, HID], BF16, kind="ExternalInput")
    y = nc.dram_tensor("y", [TPC, HID], F32, kind="ExternalOutput")

    with tile.TileContext(nc) as tc, ExitStack() as ctx:
        w_pool = ctx.enter_context(tc.tile_pool(name="w", bufs=1))
        xt_pool = ctx.enter_context(tc.tile_pool(name="xt", bufs=3))
        ysb_pool = ctx.enter_context(tc.tile_pool(name="ysb", bufs=2))
        qkv_ps_pool = ctx.enter_context(
            tc.tile_pool(name="qkvps", bufs=3, space="PSUM"))
        y_ps_pool = ctx.enter_context(tc.tile_pool(name="yps", bufs=3, space="PSUM"))
        qsb_pool = ctx.enter_context(tc.tile_pool(name="qsb", bufs=2))
        sm_pool = ctx.enter_context(tc.tile_pool(name="sm", bufs=2))
        wt_pool = ctx.enter_context(tc.tile_pool(name="wt", bufs=2))
        obf_pool = ctx.enter_context(tc.tile_pool(name="obf", bufs=2))
        ot_pool = ctx.enter_context(tc.tile_pool(name="ot", bufs=2))

        xt_sb = [None] * NTT
        obf_sb = [None] * NTT
        ot_sb = [None] * NTT

        def load_x(t):
            xt_sb[t] = xt_pool.tile([P, HID], BF16, name="xt", tag="xt")
            nc.sync.dma_start(xt_sb[t][:], xt[t * P:(t + 1) * P, :])

        # DMA emission order is the serial-DMA schedule: x0/x1, then qkv
        # weights in prologue consumption order (two halves of three 512-col
        # groups), x2/x3, then wo (first needed 3 pipeline steps in).
        xt_sb[0] = xt_pool.tile([P, HID], BF16, name="xt", tag="xt")
        nc.sync.dma_start(xt_sb[0][:, 0:2 * P], xt[0:P, 0:2 * P])
        nc.sync.dma_start(xt_sb[0][:, 2 * P:], xt[0:P, 2 * P:])
        wqa = w_pool.tile([P, NI * HID], BF16, tag="wqa", name="wqa")
        wka = w_pool.tile([P, NI * KV], BF16, tag="wka", name="wka")
        wva = w_pool.tile([P, NI * KV], BF16, tag="wva", name="wva")
        woa = w_pool.tile([P, NI * HID], BF16, tag="woa", name="woa")
        wq_sb = [wqa[:, i * HID:(i + 1) * HID] for i in range(NI)]
        wk_sb = [wka[:, i * KV:(i + 1) * KV] for i in range(NI)]
        wv_sb = [wva[:, i * KV:(i + 1) * KV] for i in range(NI)]
        wo_sb = [woa[:, i * HID:(i + 1) * HID] for i in range(NI)]
        def wchunk(dst_all, src, i0, i1, c0, c1, w_):
            # weight blocks i0..i1-1, cols c0:c1, one DMA
            nc.sync.dma_start(
                dst_all[:, i0 * w_:i1 * w_]
                .rearrange("p (i c) -> p i c", c=w_)[:, :, c0:c1],
                src[i0 * P:i1 * P, c0:c1]
                .rearrange("(i p) c -> p i c", p=P),
            )

        # small first chunk so the PE can start ~3us earlier; xt1 lands
        # right after it (tile 1's chains trail tile 0's by a few matmuls)
        HEAD = [(0, 2), (2, 4), (4, 8), (8, 12), (12, 16)]
        wchunk(wqa, wq, 0, 2, 0, 1024, HID)
        wchunk(wka, wk, 0, 2, 0, KV, KV)
        load_x(1)
        for i0, i1 in HEAD[1:]:
            wchunk(wqa, wq, i0, i1, 0, 1024, HID)
            wchunk(wka, wk, i0, i1, 0, KV, KV)
        for i0, i1 in HEAD:
            wchunk(wqa, wq, i0, i1, 1024, 2048, HID)
            wchunk(wva, wv, i0, i1, 0, KV, KV)
        load_x(2)
        load_x(3)
        for j in range(4):
            wchunk(woa, wo, 4 * j, 4 * (j + 1), 0, 1024, HID)
        for j in range(4):
            wchunk(woa, wo, 4 * j, 4 * (j + 1), 1024, 2048, HID)

        def qkv_rhs(s_, i):
            if s_ < 4:
                return wq_sb[i][:, s_ * 512:(s_ + 1) * 512]
            if s_ == 4:
                return wk_sb[i][:]
            return wv_sb[i][:]

        def attn_middle(t, qsb, ksb, vsb):
            """scores + softmax + weighted sum for tile t (DVE + ACT)."""
            sc = sm_pool.tile([P, H * G], F32, tag="sc", name="sc")
            ex = sm_pool.tile([P, H * G], F32, tag="ex", name="ex")
            dn = sm_pool.tile([P, H], F32, tag="dn", name="dn")
            rc = sm_pool.tile([P, H], F32, tag="rc", name="rc")
            pf = sm_pool.tile([P, H * G], F32, tag="pf", name="pf")
            junk = sm_pool.tile([P, D], BF16, tag="junk", name="junk")

            # raw scores sc[t,(h,g)] = <q_h, k_g>  (fused mult+reduce, DVE)
            for h in range(H):
                for g in range(G):
                    nc.vector.scalar_tensor_tensor(
                        junk[:],
                        qsb[:, h * D:(h + 1) * D],
                        1.0,
                        ksb[:, g * D:(g + 1) * D],
                        op0=mybir.AluOpType.mult,
                        op1=mybir.AluOpType.mult,
                        accum_out=sc[:, ds(h * G + g, 1)],
                    )

            # softmax over g (1/sqrt(D) folded into exp's scale)
            nc.scalar.activation(
                ex[:], sc[:], mybir.ActivationFunctionType.Exp, scale=SCALE)
            nc.vector.reduce_sum(
                dn[:], ex[:].rearrange("p (h g) -> p h g", g=G),
                axis=mybir.AxisListType.X,
            )
            nc.vector.reciprocal(rc[:], dn[:])
            nc.vector.scalar_tensor_tensor(
                pf[:].rearrange("p (h g) -> p h g", g=G),
                ex[:].rearrange("p (h g) -> p h g", g=G),
                1.0,
                rc[:].unsqueeze(2).broadcast_to((P, H, G)),
                op0=mybir.AluOpType.mult, op1=mybir.AluOpType.mult,
            )

            # o[t,(h,d)] = sum_g p[t,(h,g)] * v[t,(g,d)]  (DVE chain)
            obf = obf_pool.tile([P, HID], BF16, name="obf", tag="obf")
            obf_sb[t] = obf
            ta = wt_pool.tile([P, D], BF16, tag="ta", name="ta")
            tb = wt_pool.tile([P, D], BF16, tag="tb", name="tb")
            ab = [ta, tb]
            for h in range(H):
                nc.vector.tensor_scalar_mul(
                    ab[0][:], vsb[:, 0:D], pf[:, ds(h * G, 1)])
                for g in range(1, G):
                    dst = obf[:, h * D:(h + 1) * D] if g == G - 1 else ab[g % 2][:]
                    nc.vector.scalar_tensor_tensor(
                        dst,
                        vsb[:, g * D:(g + 1) * D],
                        pf[:, ds(h * G + g, 1)],
                        ab[(g - 1) % 2][:],
                        op0=mybir.AluOpType.mult,
                        op1=mybir.AluOpType.add,
                    )

        def transpose_o(t):
            # O^T via the DMA xbar (keeps the PE matmul-only):
            # ot[p, o*128+tok] = obf[tok, o*128+p]
            ot = ot_pool.tile([P, HID], BF16, name="ot", tag="ot")
            ot_sb[t] = ot
            nc.sync.dma_start_transpose(
                ot[:].rearrange("p (o t2) -> p o t2", t2=P), obf_sb[t][:])
            obf_sb[t] = None

        # ---- prologue: tiles 0+1 consume the qkv weight stream in arrival
        # order - six interleaved psum chains (3 from the qkv pool, 3
        # borrowed from the still-idle y pool) per half ----
        pro_sb = {}
        for tt in (0, 1):
            pro_sb[tt] = (
                qsb_pool.tile([P, HID], BF16, tag="q", name="q"),
                qsb_pool.tile([P, KV], BF16, tag="k", name="k"),
                qsb_pool.tile([P, KV], BF16, tag="v", name="v"),
            )
        for half in range(2):
            svals = (0, 1, 4) if half == 0 else (2, 3, 5)
            pss = {}
            for tt in (0, 1):
                pool = qkv_ps_pool if tt == 0 else y_ps_pool
                tagname = "ps" if tt == 0 else "yps"
                for s_ in svals:
                    pss[(tt, s_)] = pool.tile(
                        [P, 512], F32, name=tagname, tag=tagname)
            for i in range(NI):
                for tt in (0, 1):
                    for s_ in svals:
                        nc.tensor.matmul(
                            pss[(tt, s_)][:],
                            xt_sb[tt][:, i * P:(i + 1) * P],
                            qkv_rhs(s_, i),
                            start=(i == 0), stop=(i == NI - 1),
                        )
            for tt in (0, 1):
                qsb, ksb, vsb = pro_sb[tt]
                for s_ in svals:
                    ps = pss[(tt, s_)]
                    if s_ < 4:
                        nc.scalar.copy(qsb[:, s_ * 512:(s_ + 1) * 512], ps[:])
                    elif s_ == 4:
                        nc.scalar.copy(ksb[:], ps[:])
                    else:
                        nc.scalar.copy(vsb[:], ps[:])
        for tt in (0, 1):
            attn_middle(tt, *pro_sb[tt])
            transpose_o(tt)

        # ---- steady state: QKV(t) | Wo(t-3), software pipeline depth 3 ----
        for t in range(2, NTT + 3):
            if 2 <= t <= 13:
                load_x(t + 2)

            if t < NTT:
                xts = xt_sb[t]
                qsb = qsb_pool.tile([P, HID], BF16, tag="q")
                ksb = qsb_pool.tile([P, KV], BF16, tag="k")
                vsb = qsb_pool.tile([P, KV], BF16, tag="v")

                # 3 pair-interleaved group chains per tile; each xt block is
                # stationary for 2 back-to-back matmuls
                for sa, sb in ((0, 4), (1, 2), (3, 5)):
                    psa = qkv_ps_pool.tile([P, 512], F32, name="ps", tag="ps")
                    psb = qkv_ps_pool.tile([P, 512], F32, name="ps", tag="ps")
                    for i in range(NI):
                        for ps, s_ in ((psa, sa), (psb, sb)):
                            nc.tensor.matmul(
                                ps[:], xts[:, i * P:(i + 1) * P], qkv_rhs(s_, i),
                                start=(i == 0), stop=(i == NI - 1),
                            )
                    for ps, s_ in ((psa, sa), (psb, sb)):
                        if s_ < 4:
                            nc.scalar.copy(qsb[:, s_ * 512:(s_ + 1) * 512], ps[:])
                        elif s_ == 4:
                            nc.scalar.copy(ksb[:], ps[:])
                        else:
                            nc.scalar.copy(vsb[:], ps[:])

                attn_middle(t, qsb, ksb, vsb)

            # Wo matmuls + y out for tile t-3
            if t - 3 >= 0:
                tw = t - 3
                ot = ot_sb[tw]
                if tw == NTT - 1:
                    # last tile: sequential chains, final one narrow, so the
                    # post-matmul drain holds only one short copy+DMA
                    for c0, c1 in ((0, 512), (512, 1024), (1024, 1536),
                                   (1536, 1792), (1792, 2048)):
                        yps = y_ps_pool.tile([P, c1 - c0], F32,
                                             name="yps", tag="yps")
                        for o in range(NI):
                            nc.tensor.matmul(
                                yps[:],
                                ot[:, o * P:(o + 1) * P],
                                wo_sb[o][:, c0:c1],
                                start=(o == 0), stop=(o == NI - 1),
                            )
                        ysb = ysb_pool.tile([P, c1 - c0], F32,
                                            name="ysb", tag="ysb")
                        nc.scalar.copy(ysb[:], yps[:])
                        nc.sync.dma_start(
                            y[tw * P:(tw + 1) * P, c0:c1], ysb[:])
                    ot_sb[tw] = None
                    continue
                for sp in range(2):
                    ypa = y_ps_pool.tile([P, 512], F32, name="yps", tag="yps")
                    ypb = y_ps_pool.tile([P, 512], F32, name="yps", tag="yps")
                    for o in range(NI):
                        for yps, s_ in ((ypa, 2 * sp), (ypb, 2 * sp + 1)):
                            nc.tensor.matmul(
                                yps[:],
                                ot[:, o * P:(o + 1) * P],
                                wo_sb[o][:, s_ * 512:(s_ + 1) * 512],
                                start=(o == 0), stop=(o == NI - 1),
                            )
                    for yps, s_ in ((ypa, 2 * sp), (ypb, 2 * sp + 1)):
                        ysb = ysb_pool.tile([P, 512], F32, name="ysb", tag="ysb")
                        nc.scalar.copy(ysb[:], yps[:])
                        nc.sync.dma_start(
                            y[tw * P:(tw + 1) * P, s_ * 512:(s_ + 1) * 512],
                            ysb[:])
                ot_sb[tw] = None

            if 2 <= t < NTT:
                transpose_o(t)

    nc.compile()
    return nc


def _build_bias(has_bias: bool = True) -> bass.Bass:
    """Original (slower) path, kept for the biased case."""
    nc = bacc.Bacc("TRN2")
    x = nc.dram_tensor("x", [TPC, HID], BF16, kind="ExternalInput")
    wq = nc.dram_tensor("wq", [HID, HID], BF16, kind="ExternalInput")
    wk = nc.dram_tensor("wk", [HID, KV], BF16, kind="ExternalInput")
    wv = nc.dram_tensor("wv", [HID, KV], BF16, kind="ExternalInput")
    wo = nc.dram_tensor("wo", [HID, HID], BF16, kind="ExternalInput")
    if has_bias:
        bqkv = nc.dram_tensor("bqkv", [1, HID + 2 * KV], F32, kind="ExternalInput")
        bo = nc.dram_tensor("bo", [1, HID], F32, kind="ExternalInput")
    y = nc.dram_tensor("y", [TPC, HID], F32, kind="ExternalOutput")

    with tile.TileContext(nc) as tc, ExitStack() as ctx:
        const_pool = ctx.enter_context(tc.tile_pool(name="const", bufs=1))
        ident = const_pool.tile([P, P], BF16)
        make_identity(nc, ident[:])

        if has_bias:
            bias_qkv = const_pool.tile([P, HID + 2 * KV], F32)
            nc.sync.dma_start(bias_qkv[:], bqkv[0:1, :].broadcast_to((P, HID + 2 * KV)))
            bias_o = const_pool.tile([P, HID], F32)
            nc.sync.dma_start(bias_o[:], bo[0:1, :].broadcast_to((P, HID)))

        # O^T staging for the whole core: [o_block(16) x tokens(2048)] bf16
        ofm_pool = ctx.enter_context(tc.tile_pool(name="ofm", bufs=1))
        ofm = ofm_pool.tile([P, NI * TPC], BF16)

        kv_pool = ctx.enter_context(tc.tile_pool(name="wkv", bufs=1))
        wk_sb = []
        wv_sb = []
        for i in range(NI):
            wk_t = kv_pool.tile([P, KV], BF16, tag=f"wk{i}")
            nc.sync.dma_start(wk_t[:], wk[i * P:(i + 1) * P, :])
            wk_sb.append(wk_t)
            wv_t = kv_pool.tile([P, KV], BF16, tag=f"wv{i}")
            nc.sync.dma_start(wv_t[:], wv[i * P:(i + 1) * P, :])
            wv_sb.append(wv_t)

        pt_pool = ctx.enter_context(tc.tile_pool(name="pt", bufs=2, space="PSUM"))
        mm_pool = ctx.enter_context(tc.tile_pool(name="mm", bufs=3, space="PSUM"))

        # ---------------- Phase A: QKV projections + attention ----------------
        with tc.tile_pool(name="wqp", bufs=1) as wq_pool, \
             tc.tile_pool(name="xt", bufs=3) as xt_pool, \
             tc.tile_pool(name="xfm", bufs=1) as xfm_pool, \
             tc.tile_pool(name="qkv", bufs=1) as qkv_pool, \
             tc.tile_pool(name="attn", bufs=2) as attn_pool, \
             tc.tile_pool(name="oacc", bufs=1) as oacc_pool, \
             tc.tile_pool(name="obf", bufs=1) as obf_pool:
            wq_sb = []
            for i in range(NI):
                wq_t = wq_pool.tile([P, HID], BF16, tag=f"wq{i}")
                nc.sync.dma_start(wq_t[:], wq[i * P:(i + 1) * P, :])
                wq_sb.append(wq_t)

            for t in range(NTT):
                xt = xt_pool.tile([P, HID], BF16)
                nc.sync.dma_start(xt[:], x[t * P:(t + 1) * P, :])

                # transpose X tile to feature-major [i, t] (16 blocks of 128x128)
                xfm = xfm_pool.tile([P, HID], BF16)
                for j in range(4):
                    pt = pt_pool.tile([P, 512], BF16)
                    for k in range(4):
                        blk = 4 * j + k
                        nc.tensor.transpose(
                            pt[:, k * P:(k + 1) * P],
                            xt[:, blk * P:(blk + 1) * P],
                            ident[:],
                        )
                    nc.vector.tensor_copy(xfm[:, j * 512:(j + 1) * 512], pt[:])

                # QKV projections, token-major out: [t(128part), 3072]
                qkv = qkv_pool.tile([P, HID + 2 * KV], F32)
                for s in range(6):
                    ps = mm_pool.tile([P, 512], F32)
                    for i in range(NI):
                        if s < 4:
                            rhs = wq_sb[i][:, s * 512:(s + 1) * 512]
                        elif s == 4:
                            rhs = wk_sb[i][:]
                        else:
                            rhs = wv_sb[i][:]
                        nc.tensor.matmul(
                            ps[:], xfm[:, i * P:(i + 1) * P], rhs,
                            start=(i == 0), stop=(i == NI - 1),
                        )
                    if has_bias:
                        nc.vector.tensor_add(
                            qkv[:, s * 512:(s + 1) * 512], ps[:],
                            bias_qkv[:, s * 512:(s + 1) * 512],
                        )
                    else:
                        nc.vector.tensor_copy(qkv[:, s * 512:(s + 1) * 512], ps[:])

                # scores[t, h, g] = <q_h, k_g> * SCALE   (fused mult+reduce)
                sc = attn_pool.tile([P, H * G], F32, tag="sc")
                junk = attn_pool.tile([P, D], F32, tag="junk")
                for h in range(H):
                    for g in range(G):
                        nc.vector.scalar_tensor_tensor(
                            junk[:],
                            qkv[:, h * D:(h + 1) * D],
                            SCALE,
                            qkv[:, HID + g * D:HID + (g + 1) * D],
                            op0=mybir.AluOpType.mult,
                            op1=mybir.AluOpType.mult,
                            accum_out=sc[:, ds(h * G + g, 1)],
                        )

                # softmax over g (4); denominator folded into final scale
                ex = attn_pool.tile([P, H * G], F32, tag="ex")
                nc.scalar.activation(ex[:], sc[:], mybir.ActivationFunctionType.Exp)
                dn = attn_pool.tile([P, H], F32, tag="dn")
                nc.vector.reduce_sum(
                    dn[:], ex[:].rearrange("p (h g) -> p h g", g=G),
                    axis=mybir.AxisListType.X,
                )
                rc = attn_pool.tile([P, H], F32, tag="rc")
                nc.vector.reciprocal(rc[:], dn[:])

                # o[t, h*D+d] = (sum_g ex[t,h,g] * v[t, g*D+d]) * rc[t,h]
                acc = oacc_pool.tile([P, HID], F32, tag="acc")
                tmp = oacc_pool.tile([P, HID], F32, tag="tmp")
                obf = obf_pool.tile([P, HID], BF16)
                ab = [acc, tmp]
                for h in range(H):
                    hs = ds(h * D, D)
                    nc.vector.tensor_scalar_mul(
                        ab[0][:, hs],
                        qkv[:, HID + KV:HID + KV + D],
                        ex[:, ds(h * G, 1)],
                    )
                    for g in range(1, G):
                        nc.vector.scalar_tensor_tensor(
                            ab[g % 2][:, hs],
                            qkv[:, HID + KV + g * D:HID + KV + (g + 1) * D],
                            ex[:, ds(h * G + g, 1)],
                            ab[(g - 1) % 2][:, hs],
                            op0=mybir.AluOpType.mult,
                            op1=mybir.AluOpType.add,
                        )
                    nc.vector.tensor_scalar_mul(
                        obf[:, hs], ab[(G - 1) % 2][:, hs], rc[:, ds(h, 1)])

                # transpose O tile into ofm [o_block, token]
                for j in range(4):
                    pt = pt_pool.tile([P, 512], BF16)
                    for k in range(4):
                        blk = 4 * j + k
                        nc.tensor.transpose(
                            pt[:, k * P:(k + 1) * P],
                            obf[:, blk * P:(blk + 1) * P],
                            ident[:],
                        )
                    nc.vector.tensor_copy(
                        ofm[:].rearrange("p (o t) -> p o t", t=TPC)
                              [:, 4 * j:4 * j + 4, t * P:(t + 1) * P],
                        pt[:].rearrange("p (o t) -> p o t", t=P),
                    )

        # ---------------- Phase B: output projection ----------------
        with tc.tile_pool(name="wop", bufs=1) as wo_pool, \
             tc.tile_pool(name="yt", bufs=3) as yt_pool:
            wo_sb = []
            for i in range(NI):
                wo_t = wo_pool.tile([P, HID], BF16, tag=f"wo{i}")
                nc.sync.dma_start(wo_t[:], wo[i * P:(i + 1) * P, :])
                wo_sb.append(wo_t)

            for t in range(NTT):
                for s in range(4):
                    ps = mm_pool.tile([P, 512], F32)
                    for o in range(NI):
                        nc.tensor.matmul(
                            ps[:],
                            ofm[:, ds(o * TPC + t * P, P)],
                            wo_sb[o][:, s * 512:(s + 1) * 512],
                            start=(o == 0), stop=(o == NI - 1),
                        )
                    yt = yt_pool.tile([P, 512], F32)
                    if has_bias:
                        nc.vector.tensor_add(
                            yt[:], ps[:], bias_o[:, s * 512:(s + 1) * 512])
                    else:
                        nc.vector.tensor_copy(yt[:], ps[:])
                    nc.sync.dma_start(
                        y[t * P:(t + 1) * P, s * 512:(s + 1) * 512], yt[:])

    nc.compile()
    return nc


def _build(has_bias: bool) -> bass.Bass:
    return _build_bias(True) if has_bias else _build_fast()


def kernel(hidden_states, Wq, bq, Wk, bk, Wv, bv, Wo, bo, _profile=None):
    has_bias = bool(np.any(bq) or np.any(bk) or np.any(bv) or np.any(bo))
    key = has_bias
    if key not in _cache:
        _cache[key] = _build(has_bias)
    nc = _cache[key]

    bf = ml_dtypes.bfloat16
    x_flat = np.ascontiguousarray(
        np.asarray(hidden_states, dtype=np.float32).reshape(NTOK, HID)).astype(bf)
    wq_b = np.asarray(Wq, dtype=np.float32).astype(bf)
    wk_b = np.asarray(Wk, dtype=np.float32).astype(bf)
    wv_b = np.asarray(Wv, dtype=np.float32).astype(bf)
    wo_b = np.asarray(Wo, dtype=np.float32).astype(bf)

    in_maps = []
    for c in range(NCORES):
        xc = x_flat[c * TPC:(c + 1) * TPC]
        if has_bias:
            m = {
                "x": np.ascontiguousarray(xc),
                "wq": wq_b, "wk": wk_b, "wv": wv_b, "wo": wo_b,
                "bqkv": np.concatenate([
                    np.asarray(bq, np.float32), np.asarray(bk, np.float32),
                    np.asarray(bv, np.float32)]).reshape(1, HID + 2 * KV),
                "bo": np.asarray(bo, np.float32).reshape(1, HID),
            }
        else:
            # host pre-transpose: row (t*128+p), col (i*128+tok) <- x[(t,tok),(i,p)]
            xth = np.ascontiguousarray(
                xc.reshape(NTT, P, NI, P).transpose(0, 3, 2, 1).reshape(TPC, HID))
            m = {"xt": xth, "wq": wq_b, "wk": wk_b, "wv": wv_b, "wo": wo_b}
        in_maps.append(m)

    kwargs = dict(_profile) if _profile else {}
    kwargs.pop("result", None)
    res = run_bass_kernel_spmd(nc, in_maps, list(range(NCORES)), **kwargs)
    out = np.concatenate([r["y"] for r in res.results], axis=0)
    if _profile is not None:
        _profile["result"] = res
    return out.reshape(B, S, HID).astype(np.float32)



# revision 2
# speedup vs baseline: 1.2225x; 1.2225x over previous
"""Trainium2 Bass kernel for per-position grouped-query attention.

Reference computation (B=4, S=4096, HID=2048, H=16, G=4, D=128, KV=512):
    q = x @ Wq + bq ; k = x @ Wk + bk ; v = x @ Wv + bv
    scores[t,h,g] = <q[t,h,:], k[t,g,:]> / sqrt(D)     (same-position only)
    probs = softmax_g(scores)
    o[t,h,:] = sum_g probs[t,h,g] * v[t,g,:]
    y = o @ Wo + bo

Strategy: data-parallel over the 16384 flattened tokens -> 2048 tokens/core
on 8 cores, all weights replicated, no collectives.  The matmuls run as
fp8-e4m3 DoubleRow (2 contraction blocks per instruction, 0.5 cyc/row ->
4x bf16 MAC rate), with compensated splits to stay inside the 2e-2 gate:
  - x is shipped as an fp8 (hi, lo) pair: xl = fp8(x - fp8(x)).
  - Q/K projections: (xh + xl) @ fp8(512 W)  - 2 chain segments each; the
    remaining weight-quantization noise only reaches the output through the
    4-way softmax, measured ~1.1e-2 end to end.
  - V projection: xh@Wvh + xl@Wvh + xh@Wvl  (weights split hi/lo) - ~exact.
  - attention middle on DVE/ACT exactly as before (bf16 staging, ~0.1%).
  - O^T (bf16, via DMA-xbar transpose) is split on-chip into fp8 hi/lo
    (ACT cast + DVE subtract) and o @ Wo runs the same 3-chain compensated
    form.  Scale bookkeeping: weights x512, o x16, exp scale /512^2,
    y copy /8192.
Per tile the PE does 51200 cycles (vs 81920 bf16) -> ~341us across 16
tiles; weight DMA (15 MiB fp8) overlaps the 2-tile prologue like before.
"""

import os
import sys

import numpy as np

sys.path.insert(0, "/opt/trn_rl_repo")

import ml_dtypes  # noqa: E402
from contextlib import ExitStack  # noqa: E402

import concourse.bass as bass  # noqa: E402
import concourse.bacc as bacc  # noqa: E402
import concourse.mybir as mybir  # noqa: E402
import concourse.tile as tile  # noqa: E402
from concourse.bass import ds  # noqa: E402
from concourse.bass_utils import run_bass_kernel_spmd  # noqa: E402
from concourse.masks import make_identity  # noqa: E402

B, S, HID = 4, 4096, 2048
H, G = 16, 4
D = HID // H          # 128
KV = HID * G // H     # 512
NCORES = 8
NTOK = B * S          # 16384
TPC = NTOK // NCORES  # 2048 tokens per core
P = 128
NTT = TPC // P        # 16 token tiles per core
NI = HID // P         # 16 input-feature blocks
NSP = NI // 2         # 8 DoubleRow step-pairs over the contraction
SCALE = 1.0 / float(np.sqrt(D))
WS = 512.0            # weight fp8 scale
OS = 16.0             # o fp8 scale

BF16 = mybir.dt.bfloat16
F32 = mybir.dt.float32
F8 = mybir.dt.float8e4
DR = mybir.MatmulPerfMode.DoubleRow

_cache = {}


def _build_fp8() -> bass.Bass:
    """No-bias fast path: fp8 DoubleRow matmuls with compensated splits."""
    nc = bacc.Bacc("TRN2")
    # xh/xl: host-pretransposed per token tile: row (t*128+p), col (i*128+tok)
    # holds x[t*128+tok, i*128+p]  -> per tile a plain [128, 2048] slice whose
    # block i is the lhsT [feat-in-block, token] for the QKV matmuls.
    xh = nc.dram_tensor("xh", [TPC, HID], F8, kind="ExternalInput")
    xl = nc.dram_tensor("xl", [TPC, HID], F8, kind="ExternalInput")
    wq = nc.dram_tensor("wq", [HID, HID], F8, kind="ExternalInput")
    wk = nc.dram_tensor("wk", [HID, KV], F8, kind="ExternalInput")
    wvh = nc.dram_tensor("wvh", [HID, KV], F8, kind="ExternalInput")
    wvl = nc.dram_tensor("wvl", [HID, KV], F8, kind="ExternalInput")
    woh = nc.dram_tensor("woh", [HID, HID], F8, kind="ExternalInput")
    wol = nc.dram_tensor("wol", [HID, HID], F8, kind="ExternalInput")
    y = nc.dram_tensor("y", [TPC, HID], F32, kind="ExternalOutput")

    with tile.TileContext(nc) as tc, ExitStack() as ctx:
        w_pool = ctx.enter_context(tc.tile_pool(name="w", bufs=1))
        xh_pool = ctx.enter_context(tc.tile_pool(name="xh", bufs=3))
        xl_pool = ctx.enter_context(tc.tile_pool(name="xl", bufs=3))
        ysb_pool = ctx.enter_context(tc.tile_pool(name="ysb", bufs=2))
        qkv_ps_pool = ctx.enter_context(
            tc.tile_pool(name="qkvps", bufs=3, space="PSUM"))
        y_ps_pool = ctx.enter_context(tc.tile_pool(name="yps", bufs=3, space="PSUM"))
        qsb_pool = ctx.enter_context(tc.tile_pool(name="qsb", bufs=2))
        sm_pool = ctx.enter_context(tc.tile_pool(name="sm", bufs=2))
        wt_pool = ctx.enter_context(tc.tile_pool(name="wt", bufs=2))
        obf_pool = ctx.enter_context(tc.tile_pool(name="obf", bufs=2))
        ot_pool = ctx.enter_context(tc.tile_pool(name="ot", bufs=3))
        oth_pool = ctx.enter_context(tc.tile_pool(name="oth", bufs=3))
        otl_pool = ctx.enter_context(tc.tile_pool(name="otl", bufs=3))

        xh_sb = [None] * NTT
        xl_sb = [None] * NTT
        obf_sb = [None] * NTT
        ot_sb = [None] * NTT
        oth_sb = [None] * NTT
        otl_sb = [None] * NTT

        def load_xh(t):
            xh_sb[t] = xh_pool.tile([P, HID], F8, name="xh", tag="xh")
            nc.sync.dma_start(xh_sb[t][:], xh[t * P:(t + 1) * P, :])

        def load_xl(t):
            xl_sb[t] = xl_pool.tile([P, HID], F8, name="xl", tag="xl")
            nc.sync.dma_start(xl_sb[t][:], xl[t * P:(t + 1) * P, :])

        # DMA emission order is the serial-DMA schedule: weights arrive in
        # the prologue's consumption order (chunked by contraction block) so
        # the DMA-bound startup overlaps the first two tiles' compute.
        xh_sb[0] = xh_pool.tile([P, HID], F8, name="xh", tag="xh")
        nc.sync.dma_start(xh_sb[0][:, 0:2 * P], xh[0:P, 0:2 * P])
        nc.sync.dma_start(xh_sb[0][:, 2 * P:], xh[0:P, 2 * P:])

        wqa = w_pool.tile([P, NI * HID], F8, tag="wqa", name="wqa")
        wka = w_pool.tile([P, NI * KV], F8, tag="wka", name="wka")
        wvha = w_pool.tile([P, NI * KV], F8, tag="wvha", name="wvha")
        wvla = w_pool.tile([P, NI * KV], F8, tag="wvla", name="wvla")
        woha = w_pool.tile([P, NI * HID], F8, tag="woha", name="woha")
        wola = w_pool.tile([P, NI * HID], F8, tag="wola", name="wola")

        def wchunk(dst_all, src, i0, i1, c0, c1, w_):
            # weight blocks i0..i1-1, cols c0:c1, one DMA
            nc.sync.dma_start(
                dst_all[:, i0 * w_:i1 * w_]
                .rearrange("p (i c) -> p i c", c=w_)[:, :, c0:c1],
                src[i0 * P:i1 * P, c0:c1]
                .rearrange("(i p) c -> p i c", p=P),
            )

        # half-0 weights: wq cols 0:1024 + wk, in block chunks; xl0/xh1/xl1
        # land between chunks (xl needed from chain step 8, ~mid-half).
        HEAD = [(0, 2), (2, 4), (4, 8), (8, 12), (12, 16)]
        wchunk(wqa, wq, 0, 2, 0, 1024, HID)
        wchunk(wka, wk, 0, 2, 0, KV, KV)
        load_xl(0)
        load_xh(1)
        wchunk(wqa, wq, 2, 4, 0, 1024, HID)
        wchunk(wka, wk, 2, 4, 0, KV, KV)
        load_xl(1)
        for i0, i1 in HEAD[2:]:
            wchunk(wqa, wq, i0, i1, 0, 1024, HID)
            wchunk(wka, wk, i0, i1, 0, KV, KV)
        # half-1 weights: wq cols 1024:2048 + wvh; wvl trails (needed only
        # from chain step 16 of the V chains).
        for i0, i1 in HEAD:
            wchunk(wqa, wq, i0, i1, 1024, 2048, HID)
            wchunk(wvha, wvh, i0, i1, 0, KV, KV)
        for j in range(4):
            wchunk(wvla, wvl, 4 * j, 4 * (j + 1), 0, KV, KV)
        load_xh(2)
        load_xl(2)
        load_xh(3)
        load_xl(3)
        # wo hi then lo (lo consumed only in the last third of each y chain)
        for j in range(4):
            wchunk(woha, woh, 4 * j, 4 * (j + 1), 0, 1024, HID)
        for j in range(4):
            wchunk(woha, woh, 4 * j, 4 * (j + 1), 1024, 2048, HID)
        for j in range(4):
            wchunk(wola, wol, 4 * j, 4 * (j + 1), 0, 1024, HID)
        for j in range(4):
            wchunk(wola, wol, 4 * j, 4 * (j + 1), 1024, 2048, HID)

        wq_r = wqa[:].rearrange("p (i c) -> p i c", c=HID)
        wk_r = wka[:].rearrange("p (i c) -> p i c", c=KV)
        wvh_r = wvha[:].rearrange("p (i c) -> p i c", c=KV)
        wvl_r = wvla[:].rearrange("p (i c) -> p i c", c=KV)
        woh_r = woha[:].rearrange("p (i c) -> p i c", c=HID)
        wol_r = wola[:].rearrange("p (i c) -> p i c", c=HID)

        def xpair(t, s, lo):
            src = xl_sb[t] if lo else xh_sb[t]
            return src[:, 2 * s * P:(2 * s + 2) * P].rearrange(
                "p (i t2) -> p i t2", t2=P)

        def qkv_steps(t, which, c=0):
            """DoubleRow (lhsT, rhs) step list for one 512-col psum chain."""
            if which == "q":
                segs = [(False, wq_r), (True, wq_r)]
                cs = slice(c * 512, (c + 1) * 512)
            elif which == "k":
                segs = [(False, wk_r), (True, wk_r)]
                cs = slice(0, KV)
            else:  # v
                segs = [(False, wvh_r), (True, wvh_r), (False, wvl_r)]
                cs = slice(0, KV)
            steps = []
            for lo, w_r in segs:
                for s in range(NSP):
                    steps.append((xpair(t, s, lo), w_r[:, 2 * s:2 * s + 2, cs]))
            return steps

        def opair(t, s, lo):
            src = otl_sb[t] if lo else oth_sb[t]
            return src[:, 2 * s * P:(2 * s + 2) * P].rearrange(
                "p (o t2) -> p o t2", t2=P)

        def y_steps(tw, c0, c1):
            cs = slice(c0, c1)
            steps = []
            for lo, w_r in ((False, woh_r), (True, woh_r), (False, wol_r)):
                for s in range(NSP):
                    steps.append((opair(tw, s, lo), w_r[:, 2 * s:2 * s + 2, cs]))
            return steps

        def emit_chain_pair(pairs):
            """pairs: list of (psum, steps); interleave step-wise."""
            n = max(len(st) for _, st in pairs)
            for s in range(n):
                for ps, st in pairs:
                    if s < len(st):
                        lhs, rhs = st[s]
                        nc.tensor.matmul(
                            ps[:], lhs, rhs,
                            start=(s == 0), stop=(s == len(st) - 1),
                            perf_mode=DR,
                        )

        def attn_middle(t, qsb, ksb, vsb):
            """scores + softmax + weighted sum for tile t (DVE + ACT)."""
            sc = sm_pool.tile([P, H * G], F32, tag="sc", name="sc")
            ex = sm_pool.tile([P, H * G], F32, tag="ex", name="ex")
            dn = sm_pool.tile([P, H], F32, tag="dn", name="dn")
            rc = sm_pool.tile([P, H], F32, tag="rc", name="rc")
            pf = sm_pool.tile([P, H * G], F32, tag="pf", name="pf")
            junk = sm_pool.tile([P, D], BF16, tag="junk", name="junk")

            # raw scores sc[t,(h,g)] = <q_h, k_g>  (fused mult+reduce, DVE)
            for h in range(H):
                for g in range(G):
                    nc.vector.scalar_tensor_tensor(
                        junk[:],
                        qsb[:, h * D:(h + 1) * D],
                        1.0,
                        ksb[:, g * D:(g + 1) * D],
                        op0=mybir.AluOpType.mult,
                        op1=mybir.AluOpType.mult,
                        accum_out=sc[:, ds(h * G + g, 1)],
                    )

            # softmax over g; q,k carry x512 each -> exp scale /512^2
            nc.scalar.activation(
                ex[:], sc[:], mybir.ActivationFunctionType.Exp,
                scale=SCALE / (WS * WS))
            nc.vector.reduce_sum(
                dn[:], ex[:].rearrange("p (h g) -> p h g", g=G),
                axis=mybir.AxisListType.X,
            )
            nc.vector.reciprocal(rc[:], dn[:])
            # pf = OS * ex / dn  (o shipped as 16*o for the fp8 split)
            nc.vector.scalar_tensor_tensor(
                pf[:].rearrange("p (h g) -> p h g", g=G),
                ex[:].rearrange("p (h g) -> p h g", g=G),
                OS,
                rc[:].unsqueeze(2).broadcast_to((P, H, G)),
                op0=mybir.AluOpType.mult, op1=mybir.AluOpType.mult,
            )

            # o[t,(h,d)] = sum_g p[t,(h,g)] * v[t,(g,d)]  (DVE chain)
            obf = obf_pool.tile([P, HID], BF16, name="obf", tag="obf")
            obf_sb[t] = obf
            ta = wt_pool.tile([P, D], BF16, tag="ta", name="ta")
            tb = wt_pool.tile([P, D], BF16, tag="tb", name="tb")
            ab = [ta, tb]
            for h in range(H):
                nc.vector.tensor_scalar_mul(
                    ab[0][:], vsb[:, 0:D], pf[:, ds(h * G, 1)])
                for g in range(1, G):
                    dst = obf[:, h * D:(h + 1) * D] if g == G - 1 else ab[g % 2][:]
                    nc.vector.scalar_tensor_tensor(
                        dst,
                        vsb[:, g * D:(g + 1) * D],
                        pf[:, ds(h * G + g, 1)],
                        ab[(g - 1) % 2][:],
                        op0=mybir.AluOpType.mult,
                        op1=mybir.AluOpType.add,
                    )

        def transpose_split_o(t):
            # O^T via the DMA xbar (keeps the PE matmul-only):
            # ot[p, o*128+tok] = obf[tok, o*128+p]; then split to fp8 hi/lo.
            ot = ot_pool.tile([P, HID], BF16, name="ot", tag="ot")
            ot_sb[t] = ot
            nc.sync.dma_start_transpose(
                ot[:].rearrange("p (o t2) -> p o t2", t2=P), obf_sb[t][:])
            obf_sb[t] = None
            oth = oth_pool.tile([P, HID], F8, name="oth", tag="oth")
            otl = otl_pool.tile([P, HID], F8, name="otl", tag="otl")
            oth_sb[t] = oth
            otl_sb[t] = otl
            nc.scalar.copy(oth[:], ot[:])
            nc.vector.tensor_sub(otl[:], ot[:], oth[:])
            ot_sb[t] = None

        def copy_qkv(which, c, ps, qsb, ksb, vsb):
            if which == "q":
                nc.scalar.copy(qsb[:, c * 512:(c + 1) * 512], ps[:])
            elif which == "k":
                nc.scalar.copy(ksb[:], ps[:])
            else:
                nc.scalar.mul(vsb[:], ps[:], 1.0 / WS)

        # ---- prologue: tiles 0+1 consume the qkv weight stream in arrival
        # order - six interleaved psum chains (3 from the qkv pool, 3
        # borrowed from the still-idle y pool) per half ----
        pro_sb = {}
        for tt in (0, 1):
            pro_sb[tt] = (
                qsb_pool.tile([P, HID], BF16, tag="q", name="q"),
                qsb_pool.tile([P, KV], BF16, tag="k", name="k"),
                qsb_pool.tile([P, KV], BF16, tag="v", name="v"),
            )
        for half in range(2):
            chains = (("q", 0), ("q", 1), ("k", 0)) if half == 0 else \
                     (("q", 2), ("q", 3), ("v", 0))
            pairs = []
            for tt in (0, 1):
                pool = qkv_ps_pool if tt == 0 else y_ps_pool
                tagname = "ps" if tt == 0 else "yps"
                for which, c in chains:
                    ps = pool.tile([P, 512], F32, name=tagname, tag=tagname)
                    pairs.append((ps, qkv_steps(tt, which, c), tt, which, c))
            emit_chain_pair([(ps, st) for ps, st, _, _, _ in pairs])
            for ps, _, tt, which, c in pairs:
                copy_qkv(which, c, ps, *pro_sb[tt])
        for tt in (0, 1):
            attn_middle(tt, *pro_sb[tt])
            transpose_split_o(tt)

        # ---- steady state: QKV(t) | Wo(t-3), software pipeline depth 3 ----
        for t in range(2, NTT + 3):
            if 2 <= t <= 13:
                load_xh(t + 2)
                load_xl(t + 2)

            if t < NTT:
                qsb = qsb_pool.tile([P, HID], BF16, tag="q")
                ksb = qsb_pool.tile([P, KV], BF16, tag="k")
                vsb = qsb_pool.tile([P, KV], BF16, tag="v")

                # 3 pair-interleaved chains; paired chains share the
                # stationary x pair per step (back-to-back reuse)
                for pa, pb in ((("q", 0), ("k", 0)),
                               (("q", 1), ("q", 2)),
                               (("q", 3), ("v", 0))):
                    psa = qkv_ps_pool.tile([P, 512], F32, name="ps", tag="ps")
                    psb = qkv_ps_pool.tile([P, 512], F32, name="ps", tag="ps")
                    emit_chain_pair([
                        (psa, qkv_steps(t, pa[0], pa[1])),
                        (psb, qkv_steps(t, pb[0], pb[1])),
                    ])
                    copy_qkv(pa[0], pa[1], psa, qsb, ksb, vsb)
                    copy_qkv(pb[0], pb[1], psb, qsb, ksb, vsb)

                attn_middle(t, qsb, ksb, vsb)

            # Wo matmuls + y out for tile t-3
            if t - 3 >= 0:
                tw = t - 3
                if tw == NTT - 1:
                    # last tile: sequential chains, final ones narrow, so the
                    # post-matmul drain holds only one short copy+DMA
                    for c0, c1 in ((0, 512), (512, 1024), (1024, 1536),
                                   (1536, 1792), (1792, 2048)):
                        yps = y_ps_pool.tile([P, c1 - c0], F32,
                                             name="yps", tag="yps")
                        emit_chain_pair([(yps, y_steps(tw, c0, c1))])
                        ysb = ysb_pool.tile([P, c1 - c0], F32,
                                            name="ysb", tag="ysb")
                        nc.scalar.mul(ysb[:], yps[:], 1.0 / (WS * OS))
                        nc.sync.dma_start(
                            y[tw * P:(tw + 1) * P, c0:c1], ysb[:])
                    oth_sb[tw] = None
                    otl_sb[tw] = None
                    continue
                for sp in range(2):
                    ypa = y_ps_pool.tile([P, 512], F32, name="yps", tag="yps")
                    ypb = y_ps_pool.tile([P, 512], F32, name="yps", tag="yps")
                    emit_chain_pair([
                        (ypa, y_steps(tw, 2 * sp * 512, (2 * sp + 1) * 512)),
                        (ypb, y_steps(tw, (2 * sp + 1) * 512, (2 * sp + 2) * 512)),
                    ])
                    for yps, s_ in ((ypa, 2 * sp), (ypb, 2 * sp + 1)):
                        ysb = ysb_pool.tile([P, 512], F32, name="ysb", tag="ysb")
                        nc.scalar.mul(ysb[:], yps[:], 1.0 / (WS * OS))
                        nc.sync.dma_start(
                            y[tw * P:(tw + 1) * P, s_ * 512:(s_ + 1) * 512],
                            ysb[:])
                oth_sb[tw] = None
                otl_sb[tw] = None

            if 2 <= t < NTT:
                transpose_split_o(t)

    nc.compile()
    return nc


def _build_bias(has_bias: bool = True) -> bass.Bass:
    """Original (slower) path, kept for the biased case."""
    nc = bacc.Bacc("TRN2")
    x = nc.dram_tensor("x", [TPC, HID], BF16, kind="ExternalInput")
    wq = nc.dram_tensor("wq", [HID, HID], BF16, kind="ExternalInput")
    wk = nc.dram_tensor("wk", [HID, KV], BF16, kind="ExternalInput")
    wv = nc.dram_tensor("wv", [HID, KV], BF16, kind="ExternalInput")
    wo = nc.dram_tensor("wo", [HID, HID], BF16, kind="ExternalInput")
    if has_bias:
        bqkv = nc.dram_tensor("bqkv", [1, HID + 2 * KV], F32, kind="ExternalInput")
        bo = nc.dram_tensor("bo", [1, HID], F32, kind="ExternalInput")
    y = nc.dram_tensor("y", [TPC, HID], F32, kind="ExternalOutput")

    with tile.TileContext(nc) as tc, ExitStack() as ctx:
        const_pool = ctx.enter_context(tc.tile_pool(name="const", bufs=1))
        ident = const_pool.tile([P, P], BF16)
        make_identity(nc, ident[:])

        if has_bias:
            bias_qkv = const_pool.tile([P, HID + 2 * KV], F32)
            nc.sync.dma_start(bias_qkv[:], bqkv[0:1, :].broadcast_to((P, HID + 2 * KV)))
            bias_o = const_pool.tile([P, HID], F32)
            nc.sync.dma_start(bias_o[:], bo[0:1, :].broadcast_to((P, HID)))

        # O^T staging for the whole core: [o_block(16) x tokens(2048)] bf16
        ofm_pool = ctx.enter_context(tc.tile_pool(name="ofm", bufs=1))
        ofm = ofm_pool.tile([P, NI * TPC], BF16)

        kv_pool = ctx.enter_context(tc.tile_pool(name="wkv", bufs=1))
        wk_sb = []
        wv_sb = []
        for i in range(NI):
            wk_t = kv_pool.tile([P, KV], BF16, tag=f"wk{i}")
            nc.sync.dma_start(wk_t[:], wk[i * P:(i + 1) * P, :])
            wk_sb.append(wk_t)
            wv_t = kv_pool.tile([P, KV], BF16, tag=f"wv{i}")
            nc.sync.dma_start(wv_t[:], wv[i * P:(i + 1) * P, :])
            wv_sb.append(wv_t)

        pt_pool = ctx.enter_context(tc.tile_pool(name="pt", bufs=2, space="PSUM"))
        mm_pool = ctx.enter_context(tc.tile_pool(name="mm", bufs=3, space="PSUM"))

        # ---------------- Phase A: QKV projections + attention ----------------
        with tc.tile_pool(name="wqp", bufs=1) as wq_pool, \
             tc.tile_pool(name="xt", bufs=3) as xt_pool, \
             tc.tile_pool(name="xfm", bufs=1) as xfm_pool, \
             tc.tile_pool(name="qkv", bufs=1) as qkv_pool, \
             tc.tile_pool(name="attn", bufs=2) as attn_pool, \
             tc.tile_pool(name="oacc", bufs=1) as oacc_pool, \
             tc.tile_pool(name="obf", bufs=1) as obf_pool:
            wq_sb = []
            for i in range(NI):
                wq_t = wq_pool.tile([P, HID], BF16, tag=f"wq{i}")
                nc.sync.dma_start(wq_t[:], wq[i * P:(i + 1) * P, :])
                wq_sb.append(wq_t)

            for t in range(NTT):
                xt = xt_pool.tile([P, HID], BF16)
                nc.sync.dma_start(xt[:], x[t * P:(t + 1) * P, :])

                # transpose X tile to feature-major [i, t] (16 blocks of 128x128)
                xfm = xfm_pool.tile([P, HID], BF16)
                for j in range(4):
                    pt = pt_pool.tile([P, 512], BF16)
                    for k in range(4):
                        blk = 4 * j + k
                        nc.tensor.transpose(
                            pt[:, k * P:(k + 1) * P],
                            xt[:, blk * P:(blk + 1) * P],
                            ident[:],
                        )
                    nc.vector.tensor_copy(xfm[:, j * 512:(j + 1) * 512], pt[:])

                # QKV projections, token-major out: [t(128part), 3072]
                qkv = qkv_pool.tile([P, HID + 2 * KV], F32)
                for s in range(6):
                    ps = mm_pool.tile([P, 512], F32)
                    for i in range(NI):
                        if s < 4:
                            rhs = wq_sb[i][:, s * 512:(s + 1) * 512]
                        elif s == 4:
                            rhs = wk_sb[i][:]
                        else:
                            rhs = wv_sb[i][:]
                        nc.tensor.matmul(
                            ps[:], xfm[:, i * P:(i + 1) * P], rhs,
                            start=(i == 0), stop=(i == NI - 1),
                        )
                    if has_bias:
                        nc.vector.tensor_add(
                            qkv[:, s * 512:(s + 1) * 512], ps[:],
                            bias_qkv[:, s * 512:(s + 1) * 512],
                        )
                    else:
                        nc.vector.tensor_copy(qkv[:, s * 512:(s + 1) * 512], ps[:])

                # scores[t, h, g] = <q_h, k_g> * SCALE   (fused mult+reduce)
                sc = attn_pool.tile([P, H * G], F32, tag="sc")
                junk = attn_pool.tile([P, D], F32, tag="junk")
                for h in range(H):
                    for g in range(G):
                        nc.vector.scalar_tensor_tensor(
                            junk[:],
                            qkv[:, h * D:(h + 1) * D],
                            SCALE,
                            qkv[:, HID + g * D:HID + (g + 1) * D],
                            op0=mybir.AluOpType.mult,
                            op1=mybir.AluOpType.mult,
                            accum_out=sc[:, ds(h * G + g, 1)],
                        )

                # softmax over g (4); denominator folded into final scale
                ex = attn_pool.tile([P, H * G], F32, tag="ex")
                nc.scalar.activation(ex[:], sc[:], mybir.ActivationFunctionType.Exp)
                dn = attn_pool.tile([P, H], F32, tag="dn")
                nc.vector.reduce_sum(
                    dn[:], ex[:].rearrange("p (h g) -> p h g", g=G),
                    axis=mybir.AxisListType.X,
                )
                rc = attn_pool.tile([P, H], F32, tag="rc")
                nc.vector.reciprocal(rc[:], dn[:])

                # o[t, h*D+d] = (sum_g ex[t,h,g] * v[t, g*D+d]) * rc[t,h]
                acc = oacc_pool.tile([P, HID], F32, tag="acc")
                tmp = oacc_pool.tile([P, HID], F32, tag="tmp")
                obf = obf_pool.tile([P, HID], BF16)
                ab = [acc, tmp]
                for h in range(H):
                    hs = ds(h * D, D)
                    nc.vector.tensor_scalar_mul(
                        ab[0][:, hs],
                        qkv[:, HID + KV:HID + KV + D],
                        ex[:, ds(h * G, 1)],
                    )
                    for g in range(1, G):
                        nc.vector.scalar_tensor_tensor(
                            ab[g % 2][:, hs],
                            qkv[:, HID + KV + g * D:HID + KV + (g + 1) * D],
                            ex[:, ds(h * G + g, 1)],
                            ab[(g - 1) % 2][:, hs],
                            op0=mybir.AluOpType.mult,
                            op1=mybir.AluOpType.add,
                        )
                    nc.vector.tensor_scalar_mul(
                        obf[:, hs], ab[(G - 1) % 2][:, hs], rc[:, ds(h, 1)])

                # transpose O tile into ofm [o_block, token]
                for j in range(4):
                    pt = pt_pool.tile([P, 512], BF16)
                    for k in range(4):
                        blk = 4 * j + k
                        nc.tensor.transpose(
                            pt[:, k * P:(k + 1) * P],
                            obf[:, blk * P:(blk + 1) * P],
                            ident[:],
                        )
                    nc.vector.tensor_copy(
                        ofm[:].rearrange("p (o t) -> p o t", t=TPC)
                              [:, 4 * j:4 * j + 4, t * P:(t + 1) * P],
                        pt[:].rearrange("p (o t) -> p o t", t=P),
                    )

        # ---------------- Phase B: output projection ----------------
        with tc.tile_pool(name="wop", bufs=1) as wo_pool, \
             tc.tile_pool(name="yt", bufs=3) as yt_pool:
            wo_sb = []
            for i in range(NI):
                wo_t = wo_pool.tile([P, HID], BF16, tag=f"wo{i}")
                nc.sync.dma_start(wo_t[:], wo[i * P:(i + 1) * P, :])
                wo_sb.append(wo_t)

            for t in range(NTT):
                for s in range(4):
                    ps = mm_pool.tile([P, 512], F32)
                    for o in range(NI):
                        nc.tensor.matmul(
                            ps[:],
                            ofm[:, ds(o * TPC + t * P, P)],
                            wo_sb[o][:, s * 512:(s + 1) * 512],
                            start=(o == 0), stop=(o == NI - 1),
                        )
                    yt = yt_pool.tile([P, 512], F32)
                    if has_bias:
                        nc.vector.tensor_add(
                            yt[:], ps[:], bias_o[:, s * 512:(s + 1) * 512])
                    else:
                        nc.vector.tensor_copy(yt[:], ps[:])
                    nc.sync.dma_start(
                        y[t * P:(t + 1) * P, s * 512:(s + 1) * 512], yt[:])

    nc.compile()
    return nc


def _build(has_bias: bool) -> bass.Bass:
    return _build_bias(True) if has_bias else _build_fp8()


def kernel(hidden_states, Wq, bq, Wk, bk, Wv, bv, Wo, bo, _profile=None):
    has_bias = bool(np.any(bq) or np.any(bk) or np.any(bv) or np.any(bo))
    key = has_bias
    if key not in _cache:
        _cache[key] = _build(has_bias)
    nc = _cache[key]

    x_flat = np.ascontiguousarray(
        np.asarray(hidden_states, dtype=np.float32).reshape(NTOK, HID))

    in_maps = []
    if has_bias:
        bf = ml_dtypes.bfloat16
        xb = x_flat.astype(bf)
        wq_b = np.asarray(Wq, dtype=np.float32).astype(bf)
        wk_b = np.asarray(Wk, dtype=np.float32).astype(bf)
        wv_b = np.asarray(Wv, dtype=np.float32).astype(bf)
        wo_b = np.asarray(Wo, dtype=np.float32).astype(bf)
        for c in range(NCORES):
            m = {
                "x": np.ascontiguousarray(xb[c * TPC:(c + 1) * TPC]),
                "wq": wq_b, "wk": wk_b, "wv": wv_b, "wo": wo_b,
                "bqkv": np.concatenate([
                    np.asarray(bq, np.float32), np.asarray(bk, np.float32),
                    np.asarray(bv, np.float32)]).reshape(1, HID + 2 * KV),
                "bo": np.asarray(bo, np.float32).reshape(1, HID),
            }
            in_maps.append(m)
    else:
        e4 = ml_dtypes.float8_e4m3
        xh8 = x_flat.astype(e4)
        xl8 = (x_flat - xh8.astype(np.float32)).astype(e4)

        def wsplit(W):
            Wf = np.asarray(W, dtype=np.float32) * WS
            hi = Wf.astype(e4)
            lo = (Wf - hi.astype(np.float32)).astype(e4)
            return np.ascontiguousarray(hi), np.ascontiguousarray(lo)

        wq8 = np.ascontiguousarray(
            (np.asarray(Wq, np.float32) * WS).astype(e4))
        wk8 = np.ascontiguousarray(
            (np.asarray(Wk, np.float32) * WS).astype(e4))
        wvh8, wvl8 = wsplit(Wv)
        woh8, wol8 = wsplit(Wo)

        def pret(a):
            # host pre-transpose: row (t*128+p), col (i*128+tok) <- x[(t,tok),(i,p)]
            return np.ascontiguousarray(
                a.reshape(NTT, P, NI, P).transpose(0, 3, 2, 1).reshape(TPC, HID))

        for c in range(NCORES):
            m = {
                "xh": pret(xh8[c * TPC:(c + 1) * TPC]),
                "xl": pret(xl8[c * TPC:(c + 1) * TPC]),
                "wq": wq8, "wk": wk8,
                "wvh": wvh8, "wvl": wvl8,
                "woh": woh8, "wol": wol8,
            }
            in_maps.append(m)

    kwargs = dict(_profile) if _profile else {}
    kwargs.pop("result", None)
    res = run_bass_kernel_spmd(nc, in_maps, list(range(NCORES)), **kwargs)
    out = np.concatenate([r["y"] for r in res.results], axis=0)
    if _profile is not None:
        _profile["result"] = res
    return out.reshape(B, S, HID).astype(np.float32)


# revision 4
# speedup vs baseline: 1.5168x; 1.2407x over previous
"""Trainium2 Bass kernel for per-position grouped-query attention.

Reference computation (B=4, S=4096, HID=2048, H=16, G=4, D=128, KV=512):
    q = x @ Wq + bq ; k = x @ Wk + bk ; v = x @ Wv + bv
    scores[t,h,g] = <q[t,h,:], k[t,g,:]> / sqrt(D)     (same-position only)
    probs = softmax_g(scores)
    o[t,h,:] = sum_g probs[t,h,g] * v[t,g,:]
    y = o @ Wo + bo

Strategy: data-parallel over the 16384 flattened tokens -> 2048 tokens/core
on 8 cores, all weights replicated, no collectives.  The matmuls run as
fp8-e4m3 DoubleRow (2 contraction blocks per instruction, 0.5 cyc/row ->
4x bf16 MAC rate), with compensated splits to stay inside the 2e-2 gate:
  - x is shipped as an fp8 (hi, lo) pair: xl = fp8(x - fp8(x)).
  - Q/K projections: (xh + xl) @ fp8(512 W)  - 2 chain segments each; the
    remaining weight-quantization noise only reaches the output through the
    4-way softmax, measured ~1.1e-2 end to end.
  - V projection: xh@Wvh + xl@Wvh + xh@Wvl  (weights split hi/lo) - ~exact.
  - attention middle on DVE/ACT exactly as before (bf16 staging, ~0.1%).
  - O^T (bf16, via DMA-xbar transpose) is split on-chip into fp8 hi/lo
    (ACT cast + DVE subtract) and o @ Wo runs the same 3-chain compensated
    form.  Scale bookkeeping: weights x512, o x16, exp scale /512^2,
    y copy /8192.
Per tile the PE does 51200 cycles (vs 81920 bf16) -> ~341us across 16
tiles; weight DMA (15 MiB fp8) overlaps the 2-tile prologue like before.
"""

import os
import sys

import numpy as np

sys.path.insert(0, "/opt/trn_rl_repo")

import ml_dtypes  # noqa: E402
from contextlib import ExitStack  # noqa: E402

import concourse.bass as bass  # noqa: E402
import concourse.bacc as bacc  # noqa: E402
import concourse.mybir as mybir  # noqa: E402
import concourse.tile as tile  # noqa: E402
from concourse.bass import ds  # noqa: E402
from concourse.bass_utils import run_bass_kernel_spmd  # noqa: E402
from concourse.masks import make_identity  # noqa: E402

B, S, HID = 4, 4096, 2048
H, G = 16, 4
D = HID // H          # 128
KV = HID * G // H     # 512
NCORES = 8
NTOK = B * S          # 16384
TPC = NTOK // NCORES  # 2048 tokens per core
P = 128
NTT = TPC // P        # 16 token tiles per core
NI = HID // P         # 16 input-feature blocks
NSP = NI // 2         # 8 DoubleRow step-pairs over the contraction
SCALE = 1.0 / float(np.sqrt(D))
WS = 512.0            # weight fp8 scale
OS = 16.0             # o fp8 scale

BF16 = mybir.dt.bfloat16
F32 = mybir.dt.float32
F8 = mybir.dt.float8e4
DR = mybir.MatmulPerfMode.DoubleRow

_cache = {}


def _build_fp8() -> bass.Bass:
    """No-bias fast path: fp8 DoubleRow matmuls with compensated splits."""
    nc = bacc.Bacc("TRN2")
    # xh/xl: host-pretransposed per token tile: row (t*128+p), col (i*128+tok)
    # holds x[t*128+tok, i*128+p]  -> per tile a plain [128, 2048] slice whose
    # block i is the lhsT [feat-in-block, token] for the QKV matmuls.
    xh = nc.dram_tensor("xh", [TPC, HID], F8, kind="ExternalInput")
    xl = nc.dram_tensor("xl", [TPC, HID], F8, kind="ExternalInput")
    wq = nc.dram_tensor("wq", [HID, HID], F8, kind="ExternalInput")
    wk = nc.dram_tensor("wk", [HID, KV], F8, kind="ExternalInput")
    wvh = nc.dram_tensor("wvh", [HID, KV], F8, kind="ExternalInput")
    wvl = nc.dram_tensor("wvl", [HID, KV], F8, kind="ExternalInput")
    woh = nc.dram_tensor("woh", [HID, HID], F8, kind="ExternalInput")
    wol = nc.dram_tensor("wol", [HID, HID], F8, kind="ExternalInput")
    y = nc.dram_tensor("y", [TPC, HID], F32, kind="ExternalOutput")

    with tile.TileContext(nc) as tc, ExitStack() as ctx:
        w_pool = ctx.enter_context(tc.tile_pool(name="w", bufs=1))
        xh_pool = ctx.enter_context(tc.tile_pool(name="xh", bufs=3))
        xl_pool = ctx.enter_context(tc.tile_pool(name="xl", bufs=3))
        ysb_pool = ctx.enter_context(tc.tile_pool(name="ysb", bufs=2))
        qkv_ps_pool = ctx.enter_context(
            tc.tile_pool(name="qkvps", bufs=3, space="PSUM"))
        y_ps_pool = ctx.enter_context(tc.tile_pool(name="yps", bufs=3, space="PSUM"))
        qsb_pool = ctx.enter_context(tc.tile_pool(name="qsb", bufs=2))
        sm_pool = ctx.enter_context(tc.tile_pool(name="sm", bufs=2))
        wt_pool = ctx.enter_context(tc.tile_pool(name="wt", bufs=2))
        obf_pool = ctx.enter_context(tc.tile_pool(name="obf", bufs=2))
        ot_pool = ctx.enter_context(tc.tile_pool(name="ot", bufs=3))
        oth_pool = ctx.enter_context(tc.tile_pool(name="oth", bufs=3))
        otl_pool = ctx.enter_context(tc.tile_pool(name="otl", bufs=3))

        xh_sb = [None] * NTT
        xl_sb = [None] * NTT
        obf_sb = [None] * NTT
        ot_sb = [None] * NTT
        oth_sb = [None] * NTT
        otl_sb = [None] * NTT

        def load_xh(t):
            xh_sb[t] = xh_pool.tile([P, HID], F8, name="xh", tag="xh")
            nc.sync.dma_start(xh_sb[t][:], xh[t * P:(t + 1) * P, :])

        def load_xl(t):
            xl_sb[t] = xl_pool.tile([P, HID], F8, name="xl", tag="xl")
            nc.sync.dma_start(xl_sb[t][:], xl[t * P:(t + 1) * P, :])

        # DMA emission order is the serial-DMA schedule: weights arrive in
        # the prologue's consumption order (chunked by contraction block) so
        # the DMA-bound startup overlaps the first two tiles' compute.
        xh_sb[0] = xh_pool.tile([P, HID], F8, name="xh", tag="xh")
        nc.sync.dma_start(xh_sb[0][:, 0:2 * P], xh[0:P, 0:2 * P])
        nc.sync.dma_start(xh_sb[0][:, 2 * P:], xh[0:P, 2 * P:])

        wqa = w_pool.tile([P, NI * HID], F8, tag="wqa", name="wqa")
        wka = w_pool.tile([P, NI * KV], F8, tag="wka", name="wka")
        wvha = w_pool.tile([P, NI * KV], F8, tag="wvha", name="wvha")
        wvla = w_pool.tile([P, NI * KV], F8, tag="wvla", name="wvla")
        woha = w_pool.tile([P, NI * HID], F8, tag="woha", name="woha")
        wola = w_pool.tile([P, NI * HID], F8, tag="wola", name="wola")

        def wchunk(dst_all, src, i0, i1, c0, c1, w_):
            # weight blocks i0..i1-1, cols c0:c1, one DMA
            nc.sync.dma_start(
                dst_all[:, i0 * w_:i1 * w_]
                .rearrange("p (i c) -> p i c", c=w_)[:, :, c0:c1],
                src[i0 * P:i1 * P, c0:c1]
                .rearrange("(i p) c -> p i c", p=P),
            )

        # half-0 weights: wq cols 0:1024 + wk, in block chunks; xl0/xh1/xl1
        # land between chunks (xl needed from chain step 8, ~mid-half).
        HEAD = [(0, 2), (2, 4), (4, 8), (8, 12), (12, 16)]
        wchunk(wqa, wq, 0, 2, 0, 1024, HID)
        wchunk(wka, wk, 0, 2, 0, KV, KV)
        load_xl(0)
        load_xh(1)
        wchunk(wqa, wq, 2, 4, 0, 1024, HID)
        wchunk(wka, wk, 2, 4, 0, KV, KV)
        load_xl(1)
        for i0, i1 in HEAD[2:]:
            wchunk(wqa, wq, i0, i1, 0, 1024, HID)
            wchunk(wka, wk, i0, i1, 0, KV, KV)
        # half-1 weights: wq cols 1024:2048 + wvh; wvl trails (needed only
        # from chain step 16 of the V chains).
        for i0, i1 in HEAD:
            wchunk(wqa, wq, i0, i1, 1024, 2048, HID)
            wchunk(wvha, wvh, i0, i1, 0, KV, KV)
        for j in range(4):
            wchunk(wvla, wvl, 4 * j, 4 * (j + 1), 0, KV, KV)
        load_xh(2)
        load_xl(2)
        load_xh(3)
        load_xl(3)
        # wo hi then lo (lo consumed only in the last third of each y chain)
        for j in range(4):
            wchunk(woha, woh, 4 * j, 4 * (j + 1), 0, 1024, HID)
        for j in range(4):
            wchunk(woha, woh, 4 * j, 4 * (j + 1), 1024, 2048, HID)
        for j in range(4):
            wchunk(wola, wol, 4 * j, 4 * (j + 1), 0, 1024, HID)
        for j in range(4):
            wchunk(wola, wol, 4 * j, 4 * (j + 1), 1024, 2048, HID)

        wq_r = wqa[:].rearrange("p (i c) -> p i c", c=HID)
        wk_r = wka[:].rearrange("p (i c) -> p i c", c=KV)
        wvh_r = wvha[:].rearrange("p (i c) -> p i c", c=KV)
        wvl_r = wvla[:].rearrange("p (i c) -> p i c", c=KV)
        woh_r = woha[:].rearrange("p (i c) -> p i c", c=HID)
        wol_r = wola[:].rearrange("p (i c) -> p i c", c=HID)

        def xpair(t, s, lo):
            src = xl_sb[t] if lo else xh_sb[t]
            return src[:, 2 * s * P:(2 * s + 2) * P].rearrange(
                "p (i t2) -> p i t2", t2=P)

        def qkv_steps(t, which, c=0):
            """DoubleRow (lhsT, rhs) step list for one 512-col psum chain."""
            if which == "q":
                segs = [(False, wq_r), (True, wq_r)]
                cs = slice(c * 512, (c + 1) * 512)
            elif which == "k":
                segs = [(False, wk_r), (True, wk_r)]
                cs = slice(0, KV)
            else:  # v
                segs = [(False, wvh_r), (True, wvh_r), (False, wvl_r)]
                cs = slice(0, KV)
            steps = []
            for lo, w_r in segs:
                for s in range(NSP):
                    steps.append((xpair(t, s, lo), w_r[:, 2 * s:2 * s + 2, cs]))
            return steps

        def opair(t, s, lo):
            src = otl_sb[t] if lo else oth_sb[t]
            return src[:, 2 * s * P:(2 * s + 2) * P].rearrange(
                "p (o t2) -> p o t2", t2=P)

        def y_steps(tw, c0, c1):
            cs = slice(c0, c1)
            steps = []
            for lo, w_r in ((False, woh_r), (True, woh_r), (False, wol_r)):
                for s in range(NSP):
                    steps.append((opair(tw, s, lo), w_r[:, 2 * s:2 * s + 2, cs]))
            return steps

        def emit_chain_pair(pairs):
            """pairs: list of (psum, steps); interleave step-wise."""
            n = max(len(st) for _, st in pairs)
            for s in range(n):
                for ps, st in pairs:
                    if s < len(st):
                        lhs, rhs = st[s]
                        nc.tensor.matmul(
                            ps[:], lhs, rhs,
                            start=(s == 0), stop=(s == len(st) - 1),
                            perf_mode=DR,
                        )

        def attn_middle(t, qsb, ksb, vsb):
            """scores + softmax + weighted sum for tile t (DVE + ACT)."""
            sc = sm_pool.tile([P, H * G], F32, tag="sc", name="sc")
            ex = sm_pool.tile([P, H * G], F32, tag="ex", name="ex")
            dn = sm_pool.tile([P, H], F32, tag="dn", name="dn")
            rc = sm_pool.tile([P, H], F32, tag="rc", name="rc")
            pf = sm_pool.tile([P, H * G], F32, tag="pf", name="pf")
            junk = sm_pool.tile([P, D], BF16, tag="junk", name="junk")

            # raw scores sc[t,(h,g)] = <q_h, k_g>  (fused mult+reduce, DVE)
            for h in range(H):
                for g in range(G):
                    nc.vector.scalar_tensor_tensor(
                        junk[:],
                        qsb[:, h * D:(h + 1) * D],
                        1.0,
                        ksb[:, g * D:(g + 1) * D],
                        op0=mybir.AluOpType.mult,
                        op1=mybir.AluOpType.mult,
                        accum_out=sc[:, ds(h * G + g, 1)],
                    )

            # softmax over g; q,k carry x512 each -> exp scale /512^2
            nc.scalar.activation(
                ex[:], sc[:], mybir.ActivationFunctionType.Exp,
                scale=SCALE / (WS * WS))
            nc.vector.reduce_sum(
                dn[:], ex[:].rearrange("p (h g) -> p h g", g=G),
                axis=mybir.AxisListType.X,
            )
            nc.vector.reciprocal(rc[:], dn[:])
            # pf = OS * ex / dn  (o shipped as 16*o for the fp8 split)
            nc.vector.scalar_tensor_tensor(
                pf[:].rearrange("p (h g) -> p h g", g=G),
                ex[:].rearrange("p (h g) -> p h g", g=G),
                OS,
                rc[:].unsqueeze(2).broadcast_to((P, H, G)),
                op0=mybir.AluOpType.mult, op1=mybir.AluOpType.mult,
            )

            # o[t,(h,d)] = sum_g p[t,(h,g)] * v[t,(g,d)]  - batched per g on
            # the otherwise-idle GpSimd/Pool engine (4 mult + 3 accum ops of
            # [p, H*D], broadcast views; DVE keeps only the score dots)
            obf = obf_pool.tile([P, HID], BF16, name="obf", tag="obf")
            obf_sb[t] = obf
            tmp = wt_pool.tile([P, HID], BF16, tag="ta", name="ta")
            ob3 = obf[:].rearrange("p (h d) -> p h d", d=D)
            tm3 = tmp[:].rearrange("p (h d) -> p h d", d=D)
            pf3 = pf[:].rearrange("p (h g) -> p h g", g=G)
            vv = lambda g: vsb[:, g * D:(g + 1) * D].unsqueeze(1) \
                .broadcast_to((P, H, D))
            pp = lambda g: pf3[:, :, g:g + 1].broadcast_to((P, H, D))
            nc.gpsimd.tensor_tensor(ob3, vv(0), pp(0), op=mybir.AluOpType.mult)
            for g in range(1, G):
                nc.gpsimd.tensor_tensor(tm3, vv(g), pp(g),
                                        op=mybir.AluOpType.mult)
                nc.gpsimd.tensor_tensor(ob3, ob3, tm3,
                                        op=mybir.AluOpType.add)

        def transpose_split_o(t):
            # O^T via the DMA xbar (keeps the PE matmul-only):
            # ot[p, o*128+tok] = obf[tok, o*128+p]; then split to fp8 hi/lo.
            ot = ot_pool.tile([P, HID], BF16, name="ot", tag="ot")
            ot_sb[t] = ot
            nc.sync.dma_start_transpose(
                ot[:].rearrange("p (o t2) -> p o t2", t2=P), obf_sb[t][:])
            obf_sb[t] = None
            oth = oth_pool.tile([P, HID], F8, name="oth", tag="oth")
            otl = otl_pool.tile([P, HID], F8, name="otl", tag="otl")
            oth_sb[t] = oth
            otl_sb[t] = otl
            nc.scalar.copy(oth[:], ot[:])
            nc.gpsimd.tensor_sub(otl[:], ot[:], oth[:])
            ot_sb[t] = None

        def copy_qkv(which, c, ps, qsb, ksb, vsb):
            if which == "q":
                nc.scalar.copy(qsb[:, c * 512:(c + 1) * 512], ps[:])
            elif which == "k":
                nc.scalar.copy(ksb[:], ps[:])
            else:
                nc.scalar.mul(vsb[:], ps[:], 1.0 / WS)

        # ---- prologue: tiles 0+1 consume the qkv weight stream in arrival
        # order - six interleaved psum chains (3 from the qkv pool, 3
        # borrowed from the still-idle y pool) per half ----
        pro_sb = {}
        for tt in (0, 1):
            pro_sb[tt] = (
                qsb_pool.tile([P, HID], BF16, tag="q", name="q"),
                qsb_pool.tile([P, KV], BF16, tag="k", name="k"),
                qsb_pool.tile([P, KV], BF16, tag="v", name="v"),
            )
        for half in range(2):
            chains = (("q", 0), ("q", 1), ("k", 0)) if half == 0 else \
                     (("q", 2), ("q", 3), ("v", 0))
            pairs = []
            for tt in (0, 1):
                pool = qkv_ps_pool if tt == 0 else y_ps_pool
                tagname = "ps" if tt == 0 else "yps"
                for which, c in chains:
                    ps = pool.tile([P, 512], F32, name=tagname, tag=tagname)
                    pairs.append((ps, qkv_steps(tt, which, c), tt, which, c))
            emit_chain_pair([(ps, st) for ps, st, _, _, _ in pairs])
            for ps, _, tt, which, c in pairs:
                copy_qkv(which, c, ps, *pro_sb[tt])
        for tt in (0, 1):
            attn_middle(tt, *pro_sb[tt])
            transpose_split_o(tt)

        # ---- steady state: QKV(t) | Wo(t-3), software pipeline depth 3 ----
        for t in range(2, NTT + 3):
            if 2 <= t <= 13:
                load_xh(t + 2)
                load_xl(t + 2)

            if t < NTT:
                qsb = qsb_pool.tile([P, HID], BF16, tag="q")
                ksb = qsb_pool.tile([P, KV], BF16, tag="k")
                vsb = qsb_pool.tile([P, KV], BF16, tag="v")

                # 3 pair-interleaved chains; paired chains share the
                # stationary x pair per step (back-to-back reuse)
                for pa, pb in ((("q", 0), ("k", 0)),
                               (("q", 1), ("q", 2)),
                               (("q", 3), ("v", 0))):
                    psa = qkv_ps_pool.tile([P, 512], F32, name="ps", tag="ps")
                    psb = qkv_ps_pool.tile([P, 512], F32, name="ps", tag="ps")
                    emit_chain_pair([
                        (psa, qkv_steps(t, pa[0], pa[1])),
                        (psb, qkv_steps(t, pb[0], pb[1])),
                    ])
                    copy_qkv(pa[0], pa[1], psa, qsb, ksb, vsb)
                    copy_qkv(pb[0], pb[1], psb, qsb, ksb, vsb)

                attn_middle(t, qsb, ksb, vsb)

            # Wo matmuls + y out for tile t-3
            if t - 3 >= 0:
                tw = t - 3
                if tw == NTT - 1:
                    # last tile: sequential chains, final ones narrow, so the
                    # post-matmul drain holds only one short copy+DMA
                    for c0, c1 in ((0, 512), (512, 1024), (1024, 1536),
                                   (1536, 1792), (1792, 2048)):
                        yps = y_ps_pool.tile([P, c1 - c0], F32,
                                             name="yps", tag="yps")
                        emit_chain_pair([(yps, y_steps(tw, c0, c1))])
                        ysb = ysb_pool.tile([P, c1 - c0], F32,
                                            name="ysb", tag="ysb")
                        nc.scalar.mul(ysb[:], yps[:], 1.0 / (WS * OS))
                        nc.sync.dma_start(
                            y[tw * P:(tw + 1) * P, c0:c1], ysb[:])
                    oth_sb[tw] = None
                    otl_sb[tw] = None
                    continue
                for sp in range(2):
                    ypa = y_ps_pool.tile([P, 512], F32, name="yps", tag="yps")
                    ypb = y_ps_pool.tile([P, 512], F32, name="yps", tag="yps")
                    emit_chain_pair([
                        (ypa, y_steps(tw, 2 * sp * 512, (2 * sp + 1) * 512)),
                        (ypb, y_steps(tw, (2 * sp + 1) * 512, (2 * sp + 2) * 512)),
                    ])
                    for yps, s_ in ((ypa, 2 * sp), (ypb, 2 * sp + 1)):
                        ysb = ysb_pool.tile([P, 512], F32, name="ysb", tag="ysb")
                        nc.scalar.mul(ysb[:], yps[:], 1.0 / (WS * OS))
                        nc.sync.dma_start(
                            y[tw * P:(tw + 1) * P, s_ * 512:(s_ + 1) * 512],
                            ysb[:])
                oth_sb[tw] = None
                otl_sb[tw] = None

            if 2 <= t < NTT:
                transpose_split_o(t)

    nc.compile()
    return nc


def _build_bias(has_bias: bool = True) -> bass.Bass:
    """Original (slower) path, kept for the biased case."""
    nc = bacc.Bacc("TRN2")
    x = nc.dram_tensor("x", [TPC, HID], BF16, kind="ExternalInput")
    wq = nc.dram_tensor("wq", [HID, HID], BF16, kind="ExternalInput")
    wk = nc.dram_tensor("wk", [HID, KV], BF16, kind="ExternalInput")
    wv = nc.dram_tensor("wv", [HID, KV], BF16, kind="ExternalInput")
    wo = nc.dram_tensor("wo", [HID, HID], BF16, kind="ExternalInput")
    if has_bias:
        bqkv = nc.dram_tensor("bqkv", [1, HID + 2 * KV], F32, kind="ExternalInput")
        bo = nc.dram_tensor("bo", [1, HID], F32, kind="ExternalInput")
    y = nc.dram_tensor("y", [TPC, HID], F32, kind="ExternalOutput")

    with tile.TileContext(nc) as tc, ExitStack() as ctx:
        const_pool = ctx.enter_context(tc.tile_pool(name="const", bufs=1))
        ident = const_pool.tile([P, P], BF16)
        make_identity(nc, ident[:])

        if has_bias:
            bias_qkv = const_pool.tile([P, HID + 2 * KV], F32)
            nc.sync.dma_start(bias_qkv[:], bqkv[0:1, :].broadcast_to((P, HID + 2 * KV)))
            bias_o = const_pool.tile([P, HID], F32)
            nc.sync.dma_start(bias_o[:], bo[0:1, :].broadcast_to((P, HID)))

        # O^T staging for the whole core: [o_block(16) x tokens(2048)] bf16
        ofm_pool = ctx.enter_context(tc.tile_pool(name="ofm", bufs=1))
        ofm = ofm_pool.tile([P, NI * TPC], BF16)

        kv_pool = ctx.enter_context(tc.tile_pool(name="wkv", bufs=1))
        wk_sb = []
        wv_sb = []
        for i in range(NI):
            wk_t = kv_pool.tile([P, KV], BF16, tag=f"wk{i}")
            nc.sync.dma_start(wk_t[:], wk[i * P:(i + 1) * P, :])
            wk_sb.append(wk_t)
            wv_t = kv_pool.tile([P, KV], BF16, tag=f"wv{i}")
            nc.sync.dma_start(wv_t[:], wv[i * P:(i + 1) * P, :])
            wv_sb.append(wv_t)

        pt_pool = ctx.enter_context(tc.tile_pool(name="pt", bufs=2, space="PSUM"))
        mm_pool = ctx.enter_context(tc.tile_pool(name="mm", bufs=3, space="PSUM"))

        # ---------------- Phase A: QKV projections + attention ----------------
        with tc.tile_pool(name="wqp", bufs=1) as wq_pool, \
             tc.tile_pool(name="xt", bufs=3) as xt_pool, \
             tc.tile_pool(name="xfm", bufs=1) as xfm_pool, \
             tc.tile_pool(name="qkv", bufs=1) as qkv_pool, \
             tc.tile_pool(name="attn", bufs=2) as attn_pool, \
             tc.tile_pool(name="oacc", bufs=1) as oacc_pool, \
             tc.tile_pool(name="obf", bufs=1) as obf_pool:
            wq_sb = []
            for i in range(NI):
                wq_t = wq_pool.tile([P, HID], BF16, tag=f"wq{i}")
                nc.sync.dma_start(wq_t[:], wq[i * P:(i + 1) * P, :])
                wq_sb.append(wq_t)

            for t in range(NTT):
                xt = xt_pool.tile([P, HID], BF16)
                nc.sync.dma_start(xt[:], x[t * P:(t + 1) * P, :])

                # transpose X tile to feature-major [i, t] (16 blocks of 128x128)
                xfm = xfm_pool.tile([P, HID], BF16)
                for j in range(4):
                    pt = pt_pool.tile([P, 512], BF16)
                    for k in range(4):
                        blk = 4 * j + k
                        nc.tensor.transpose(
                            pt[:, k * P:(k + 1) * P],
                            xt[:, blk * P:(blk + 1) * P],
                            ident[:],
                        )
                    nc.vector.tensor_copy(xfm[:, j * 512:(j + 1) * 512], pt[:])

                # QKV projections, token-major out: [t(128part), 3072]
                qkv = qkv_pool.tile([P, HID + 2 * KV], F32)
                for s in range(6):
                    ps = mm_pool.tile([P, 512], F32)
                    for i in range(NI):
                        if s < 4:
                            rhs = wq_sb[i][:, s * 512:(s + 1) * 512]
                        elif s == 4:
                            rhs = wk_sb[i][:]
                        else:
                            rhs = wv_sb[i][:]
                        nc.tensor.matmul(
                            ps[:], xfm[:, i * P:(i + 1) * P], rhs,
                            start=(i == 0), stop=(i == NI - 1),
                        )
                    if has_bias:
                        nc.vector.tensor_add(
                            qkv[:, s * 512:(s + 1) * 512], ps[:],
                            bias_qkv[:, s * 512:(s + 1) * 512],
                        )
                    else:
                        nc.vector.tensor_copy(qkv[:, s * 512:(s + 1) * 512], ps[:])

                # scores[t, h, g] = <q_h, k_g> * SCALE   (fused mult+reduce)
                sc = attn_pool.tile([P, H * G], F32, tag="sc")
                junk = attn_pool.tile([P, D], F32, tag="junk")
                for h in range(H):
                    for g in range(G):
                        nc.vector.scalar_tensor_tensor(
                            junk[:],
                            qkv[:, h * D:(h + 1) * D],
                            SCALE,
                            qkv[:, HID + g * D:HID + (g + 1) * D],
                            op0=mybir.AluOpType.mult,
                            op1=mybir.AluOpType.mult,
                            accum_out=sc[:, ds(h * G + g, 1)],
                        )

                # softmax over g (4); denominator folded into final scale
                ex = attn_pool.tile([P, H * G], F32, tag="ex")
                nc.scalar.activation(ex[:], sc[:], mybir.ActivationFunctionType.Exp)
                dn = attn_pool.tile([P, H], F32, tag="dn")
                nc.vector.reduce_sum(
                    dn[:], ex[:].rearrange("p (h g) -> p h g", g=G),
                    axis=mybir.AxisListType.X,
                )
                rc = attn_pool.tile([P, H], F32, tag="rc")
                nc.vector.reciprocal(rc[:], dn[:])

                # o[t, h*D+d] = (sum_g ex[t,h,g] * v[t, g*D+d]) * rc[t,h]
                acc = oacc_pool.tile([P, HID], F32, tag="acc")
                tmp = oacc_pool.tile([P, HID], F32, tag="tmp")
                obf = obf_pool.tile([P, HID], BF16)
                ab = [acc, tmp]
                for h in range(H):
                    hs = ds(h * D, D)
                    nc.vector.tensor_scalar_mul(
                        ab[0][:, hs],
                        qkv[:, HID + KV:HID + KV + D],
                        ex[:, ds(h * G, 1)],
                    )
                    for g in range(1, G):
                        nc.vector.scalar_tensor_tensor(
                            ab[g % 2][:, hs],
                            qkv[:, HID + KV + g * D:HID + KV + (g + 1) * D],
                            ex[:, ds(h * G + g, 1)],
                            ab[(g - 1) % 2][:, hs],
                            op0=mybir.AluOpType.mult,
                            op1=mybir.AluOpType.add,
                        )
                    nc.vector.tensor_scalar_mul(
                        obf[:, hs], ab[(G - 1) % 2][:, hs], rc[:, ds(h, 1)])

                # transpose O tile into ofm [o_block, token]
                for j in range(4):
                    pt = pt_pool.tile([P, 512], BF16)
                    for k in range(4):
                        blk = 4 * j + k
                        nc.tensor.transpose(
                            pt[:, k * P:(k + 1) * P],
                            obf[:, blk * P:(blk + 1) * P],
                            ident[:],
                        )
                    nc.vector.tensor_copy(
                        ofm[:].rearrange("p (o t) -> p o t", t=TPC)
                              [:, 4 * j:4 * j + 4, t * P:(t + 1) * P],
                        pt[:].rearrange("p (o t) -> p o t", t=P),
                    )

        # ---------------- Phase B: output projection ----------------
        with tc.tile_pool(name="wop", bufs=1) as wo_pool, \
             tc.tile_pool(name="yt", bufs=3) as yt_pool:
            wo_sb = []
            for i in range(NI):
                wo_t = wo_pool.tile([P, HID], BF16, tag=f"wo{i}")
                nc.sync.dma_start(wo_t[:], wo[i * P:(i + 1) * P, :])
                wo_sb.append(wo_t)

            for t in range(NTT):
                for s in range(4):
                    ps = mm_pool.tile([P, 512], F32)
                    for o in range(NI):
                        nc.tensor.matmul(
                            ps[:],
                            ofm[:, ds(o * TPC + t * P, P)],
                            wo_sb[o][:, s * 512:(s + 1) * 512],
                            start=(o == 0), stop=(o == NI - 1),
                        )
                    yt = yt_pool.tile([P, 512], F32)
                    if has_bias:
                        nc.vector.tensor_add(
                            yt[:], ps[:], bias_o[:, s * 512:(s + 1) * 512])
                    else:
                        nc.vector.tensor_copy(yt[:], ps[:])
                    nc.sync.dma_start(
                        y[t * P:(t + 1) * P, s * 512:(s + 1) * 512], yt[:])

    nc.compile()
    return nc


def _build(has_bias: bool) -> bass.Bass:
    return _build_bias(True) if has_bias else _build_fp8()


def kernel(hidden_states, Wq, bq, Wk, bk, Wv, bv, Wo, bo, _profile=None):
    has_bias = bool(np.any(bq) or np.any(bk) or np.any(bv) or np.any(bo))
    key = has_bias
    if key not in _cache:
        _cache[key] = _build(has_bias)
    nc = _cache[key]

    x_flat = np.ascontiguousarray(
        np.asarray(hidden_states, dtype=np.float32).reshape(NTOK, HID))

    in_maps = []
    if has_bias:
        bf = ml_dtypes.bfloat16
        xb = x_flat.astype(bf)
        wq_b = np.asarray(Wq, dtype=np.float32).astype(bf)
        wk_b = np.asarray(Wk, dtype=np.float32).astype(bf)
        wv_b = np.asarray(Wv, dtype=np.float32).astype(bf)
        wo_b = np.asarray(Wo, dtype=np.float32).astype(bf)
        for c in range(NCORES):
            m = {
                "x": np.ascontiguousarray(xb[c * TPC:(c + 1) * TPC]),
                "wq": wq_b, "wk": wk_b, "wv": wv_b, "wo": wo_b,
                "bqkv": np.concatenate([
                    np.asarray(bq, np.float32), np.asarray(bk, np.float32),
                    np.asarray(bv, np.float32)]).reshape(1, HID + 2 * KV),
                "bo": np.asarray(bo, np.float32).reshape(1, HID),
            }
            in_maps.append(m)
    else:
        e4 = ml_dtypes.float8_e4m3
        xh8 = x_flat.astype(e4)
        xl8 = (x_flat - xh8.astype(np.float32)).astype(e4)

        def wsplit(W):
            Wf = np.asarray(W, dtype=np.float32) * WS
            hi = Wf.astype(e4)
            lo = (Wf - hi.astype(np.float32)).astype(e4)
            return np.ascontiguousarray(hi), np.ascontiguousarray(lo)

        wq8 = np.ascontiguousarray(
            (np.asarray(Wq, np.float32) * WS).astype(e4))
        wk8 = np.ascontiguousarray(
            (np.asarray(Wk, np.float32) * WS).astype(e4))
        wvh8, wvl8 = wsplit(Wv)
        woh8, wol8 = wsplit(Wo)

        def pret(a):
            # host pre-transpose: row (t*128+p), col (i*128+tok) <- x[(t,tok),(i,p)]
            return np.ascontiguousarray(
                a.reshape(NTT, P, NI, P).transpose(0, 3, 2, 1).reshape(TPC, HID))

        for c in range(NCORES):
            m = {
                "xh": pret(xh8[c * TPC:(c + 1) * TPC]),
                "xl": pret(xl8[c * TPC:(c + 1) * TPC]),
                "wq": wq8, "wk": wk8,
                "wvh": wvh8, "wvl": wvl8,
                "woh": woh8, "wol": wol8,
            }
            in_maps.append(m)

    kwargs = dict(_profile) if _profile else {}
    kwargs.pop("result", None)
    res = run_bass_kernel_spmd(nc, in_maps, list(range(NCORES)), **kwargs)
    out = np.concatenate([r["y"] for r in res.results], axis=0)
    if _profile is not None:
        _profile["result"] = res
    return out.reshape(B, S, HID).astype(np.float32)


# revision 7
# speedup vs baseline: 1.5212x; 1.0029x over previous
"""Trainium2 Bass kernel for per-position grouped-query attention.

Reference computation (B=4, S=4096, HID=2048, H=16, G=4, D=128, KV=512):
    q = x @ Wq + bq ; k = x @ Wk + bk ; v = x @ Wv + bv
    scores[t,h,g] = <q[t,h,:], k[t,g,:]> / sqrt(D)     (same-position only)
    probs = softmax_g(scores)
    o[t,h,:] = sum_g probs[t,h,g] * v[t,g,:]
    y = o @ Wo + bo

Strategy: data-parallel over the 16384 flattened tokens -> 2048 tokens/core
on 8 cores, all weights replicated, no collectives.  The matmuls run as
fp8-e4m3 DoubleRow (2 contraction blocks per instruction, 0.5 cyc/row ->
4x bf16 MAC rate), with compensated splits to stay inside the 2e-2 gate:
  - x is shipped as an fp8 (hi, lo) pair: xl = fp8(x - fp8(x)).
  - Q/K projections: (xh + xl) @ fp8(512 W)  - 2 chain segments each; the
    remaining weight-quantization noise only reaches the output through the
    4-way softmax, measured ~1.1e-2 end to end.
  - V projection: xh@Wvh + xl@Wvh + xh@Wvl  (weights split hi/lo) - ~exact.
  - attention middle on DVE/ACT exactly as before (bf16 staging, ~0.1%).
  - O^T (bf16, via DMA-xbar transpose) is split on-chip into fp8 hi/lo
    (ACT cast + DVE subtract) and o @ Wo runs the same 3-chain compensated
    form.  Scale bookkeeping: weights x512, o x16, exp scale /512^2,
    y copy /8192.
Per tile the PE does 51200 cycles (vs 81920 bf16) -> ~341us across 16
tiles; weight DMA (15 MiB fp8) overlaps the 2-tile prologue like before.
"""

import os
import sys

import numpy as np

sys.path.insert(0, "/opt/trn_rl_repo")

import ml_dtypes  # noqa: E402
from contextlib import ExitStack  # noqa: E402

import concourse.bass as bass  # noqa: E402
import concourse.bacc as bacc  # noqa: E402
import concourse.mybir as mybir  # noqa: E402
import concourse.tile as tile  # noqa: E402
from concourse.bass import ds  # noqa: E402
from concourse.bass_utils import run_bass_kernel_spmd  # noqa: E402
from concourse.masks import make_identity  # noqa: E402

B, S, HID = 4, 4096, 2048
H, G = 16, 4
D = HID // H          # 128
KV = HID * G // H     # 512
NCORES = 8
NTOK = B * S          # 16384
TPC = NTOK // NCORES  # 2048 tokens per core
P = 128
NTT = TPC // P        # 16 token tiles per core
NI = HID // P         # 16 input-feature blocks
NSP = NI // 2         # 8 DoubleRow step-pairs over the contraction
SCALE = 1.0 / float(np.sqrt(D))
WS = 512.0            # weight fp8 scale
OS = 16.0             # o fp8 scale

BF16 = mybir.dt.bfloat16
F32 = mybir.dt.float32
F8 = mybir.dt.float8e4
DR = mybir.MatmulPerfMode.DoubleRow

_cache = {}


def _build_fp8() -> bass.Bass:
    """No-bias fast path: fp8 DoubleRow matmuls with compensated splits."""
    nc = bacc.Bacc("TRN2")
    # xh/xl: host-pretransposed per token tile: row (t*128+p), col (i*128+tok)
    # holds x[t*128+tok, i*128+p]  -> per tile a plain [128, 2048] slice whose
    # block i is the lhsT [feat-in-block, token] for the QKV matmuls.
    xh = nc.dram_tensor("xh", [TPC, HID], F8, kind="ExternalInput")
    xl = nc.dram_tensor("xl", [TPC, HID], F8, kind="ExternalInput")
    wq = nc.dram_tensor("wq", [HID, HID], F8, kind="ExternalInput")
    wk = nc.dram_tensor("wk", [HID, KV], F8, kind="ExternalInput")
    wvh = nc.dram_tensor("wvh", [HID, KV], F8, kind="ExternalInput")
    wvl = nc.dram_tensor("wvl", [HID, KV], F8, kind="ExternalInput")
    woh = nc.dram_tensor("woh", [HID, HID], F8, kind="ExternalInput")
    wol = nc.dram_tensor("wol", [HID, HID], F8, kind="ExternalInput")
    y = nc.dram_tensor("y", [TPC, HID], F32, kind="ExternalOutput")

    with tile.TileContext(nc) as tc, ExitStack() as ctx:
        w_pool = ctx.enter_context(tc.tile_pool(name="w", bufs=1))
        xh_pool = ctx.enter_context(tc.tile_pool(name="xh", bufs=3))
        xl_pool = ctx.enter_context(tc.tile_pool(name="xl", bufs=3))
        ysb_pool = ctx.enter_context(tc.tile_pool(name="ysb", bufs=2))
        qkv_ps_pool = ctx.enter_context(
            tc.tile_pool(name="qkvps", bufs=3, space="PSUM"))
        y_ps_pool = ctx.enter_context(tc.tile_pool(name="yps", bufs=3, space="PSUM"))
        qsb_pool = ctx.enter_context(tc.tile_pool(name="qsb", bufs=2))
        sm_pool = ctx.enter_context(tc.tile_pool(name="sm", bufs=2))
        wt_pool = ctx.enter_context(tc.tile_pool(name="wt", bufs=1))
        obf_pool = ctx.enter_context(tc.tile_pool(name="obf", bufs=2))
        ot_pool = ctx.enter_context(tc.tile_pool(name="ot", bufs=2))
        oth_pool = ctx.enter_context(tc.tile_pool(name="oth", bufs=5))
        otl_pool = ctx.enter_context(tc.tile_pool(name="otl", bufs=5))

        xh_sb = [None] * NTT
        xl_sb = [None] * NTT
        obf_sb = [None] * NTT
        ot_sb = [None] * NTT
        oth_sb = [None] * NTT
        otl_sb = [None] * NTT

        def load_xh(t):
            xh_sb[t] = xh_pool.tile([P, HID], F8, name="xh", tag="xh")
            nc.sync.dma_start(xh_sb[t][:], xh[t * P:(t + 1) * P, :])

        def load_xl(t):
            xl_sb[t] = xl_pool.tile([P, HID], F8, name="xl", tag="xl")
            nc.sync.dma_start(xl_sb[t][:], xl[t * P:(t + 1) * P, :])

        # DMA emission order is the serial-DMA schedule: weights arrive in
        # the prologue's consumption order (chunked by contraction block) so
        # the DMA-bound startup overlaps the first two tiles' compute.
        xh_sb[0] = xh_pool.tile([P, HID], F8, name="xh", tag="xh")
        nc.sync.dma_start(xh_sb[0][:, 0:2 * P], xh[0:P, 0:2 * P])
        nc.sync.dma_start(xh_sb[0][:, 2 * P:], xh[0:P, 2 * P:])
        load_xh(1)

        wqa = w_pool.tile([P, NI * HID], F8, tag="wqa", name="wqa")
        wka = w_pool.tile([P, NI * KV], F8, tag="wka", name="wka")
        wvha = w_pool.tile([P, NI * KV], F8, tag="wvha", name="wvha")
        wvla = w_pool.tile([P, NI * KV], F8, tag="wvla", name="wvla")
        woha = w_pool.tile([P, NI * HID], F8, tag="woha", name="woha")
        wola = w_pool.tile([P, NI * HID], F8, tag="wola", name="wola")

        def wchunk(dst_all, src, i0, i1, c0, c1, w_):
            # weight blocks i0..i1-1, cols c0:c1, one DMA
            nc.sync.dma_start(
                dst_all[:, i0 * w_:i1 * w_]
                .rearrange("p (i c) -> p i c", c=w_)[:, :, c0:c1],
                src[i0 * P:i1 * P, c0:c1]
                .rearrange("(i p) c -> p i c", p=P),
            )

        # half-0 weights: wq cols 0:1024 + wk, in 2-block chunks (matching
        # the chains' pair consumption, so the PE never waits on a chunk
        # bigger than one step); xl0/xl1 land mid-stream (xl needed from
        # chain step 8, ~mid-half).
        PAIRS = [(2 * j, 2 * j + 2) for j in range(NSP)]
        wchunk(wqa, wq, 0, 2, 0, 1024, HID)
        wchunk(wka, wk, 0, 2, 0, KV, KV)
        load_xl(0)
        wchunk(wqa, wq, 2, 4, 0, 1024, HID)
        wchunk(wka, wk, 2, 4, 0, KV, KV)
        load_xl(1)
        for i0, i1 in PAIRS[2:]:
            wchunk(wqa, wq, i0, i1, 0, 1024, HID)
            wchunk(wka, wk, i0, i1, 0, KV, KV)
        # half-1 weights: wq cols 1024:2048 + wvh; wvl trails (needed only
        # from chain step 16 of the V chains).
        for i0, i1 in PAIRS[:4]:
            wchunk(wqa, wq, i0, i1, 1024, 2048, HID)
            wchunk(wvha, wvh, i0, i1, 0, KV, KV)
        load_xh(2)
        load_xl(2)
        for i0, i1 in PAIRS[4:]:
            wchunk(wqa, wq, i0, i1, 1024, 2048, HID)
            wchunk(wvha, wvh, i0, i1, 0, KV, KV)
        for i0, i1 in PAIRS[:4]:
            wchunk(wvla, wvl, i0, i1, 0, KV, KV)
        load_xh(3)
        load_xl(3)
        for i0, i1 in PAIRS[4:]:
            wchunk(wvla, wvl, i0, i1, 0, KV, KV)
        # wo hi then lo (lo consumed only in the last third of each y chain)
        for j in range(4):
            wchunk(woha, woh, 4 * j, 4 * (j + 1), 0, 1024, HID)
        for j in range(4):
            wchunk(woha, woh, 4 * j, 4 * (j + 1), 1024, 2048, HID)
        for j in range(4):
            wchunk(wola, wol, 4 * j, 4 * (j + 1), 0, 1024, HID)
        for j in range(4):
            wchunk(wola, wol, 4 * j, 4 * (j + 1), 1024, 2048, HID)

        wq_r = wqa[:].rearrange("p (i c) -> p i c", c=HID)
        wk_r = wka[:].rearrange("p (i c) -> p i c", c=KV)
        wvh_r = wvha[:].rearrange("p (i c) -> p i c", c=KV)
        wvl_r = wvla[:].rearrange("p (i c) -> p i c", c=KV)
        woh_r = woha[:].rearrange("p (i c) -> p i c", c=HID)
        wol_r = wola[:].rearrange("p (i c) -> p i c", c=HID)

        def xpair(t, s, lo):
            src = xl_sb[t] if lo else xh_sb[t]
            return src[:, 2 * s * P:(2 * s + 2) * P].rearrange(
                "p (i t2) -> p i t2", t2=P)

        def qkv_steps(t, which, c=0):
            """DoubleRow (lhsT, rhs) step list for one 512-col psum chain."""
            if which == "q":
                segs = [(False, wq_r), (True, wq_r)]
                cs = slice(c * 512, (c + 1) * 512)
            elif which == "k":
                segs = [(False, wk_r), (True, wk_r)]
                cs = slice(0, KV)
            else:  # v
                segs = [(False, wvh_r), (True, wvh_r), (False, wvl_r)]
                cs = slice(0, KV)
            steps = []
            for lo, w_r in segs:
                for s in range(NSP):
                    steps.append((xpair(t, s, lo), w_r[:, 2 * s:2 * s + 2, cs]))
            return steps

        def opair(t, s, lo):
            src = otl_sb[t] if lo else oth_sb[t]
            return src[:, 2 * s * P:(2 * s + 2) * P].rearrange(
                "p (o t2) -> p o t2", t2=P)

        def y_steps(tw, c0, c1):
            cs = slice(c0, c1)
            steps = []
            for lo, w_r in ((False, woh_r), (True, woh_r), (False, wol_r)):
                for s in range(NSP):
                    steps.append((opair(tw, s, lo), w_r[:, 2 * s:2 * s + 2, cs]))
            return steps

        def emit_chain_pair(pairs):
            """pairs: list of (psum, steps); interleave step-wise."""
            n = max(len(st) for _, st in pairs)
            for s in range(n):
                for ps, st in pairs:
                    if s < len(st):
                        lhs, rhs = st[s]
                        nc.tensor.matmul(
                            ps[:], lhs, rhs,
                            start=(s == 0), stop=(s == len(st) - 1),
                            perf_mode=DR,
                        )

        def attn_middle(t, qsb, ksb, vsb):
            """scores + softmax + weighted sum for tile t (DVE + ACT)."""
            sc = sm_pool.tile([P, H * G], F32, tag="sc", name="sc")
            ex = sm_pool.tile([P, H * G], F32, tag="ex", name="ex")
            dn = sm_pool.tile([P, H], F32, tag="dn", name="dn")
            rc = sm_pool.tile([P, H], F32, tag="rc", name="rc")
            pf = sm_pool.tile([P, H * G], F32, tag="pf", name="pf")
            junk = sm_pool.tile([P, D], BF16, tag="junk", name="junk")

            # raw scores sc[t,(h,g)] = <q_h, k_g>  (fused mult+reduce, DVE)
            for h in range(H):
                for g in range(G):
                    nc.vector.scalar_tensor_tensor(
                        junk[:],
                        qsb[:, h * D:(h + 1) * D],
                        1.0,
                        ksb[:, g * D:(g + 1) * D],
                        op0=mybir.AluOpType.mult,
                        op1=mybir.AluOpType.mult,
                        accum_out=sc[:, ds(h * G + g, 1)],
                    )

            # softmax over g; q,k carry x512 each -> exp scale /512^2
            nc.scalar.activation(
                ex[:], sc[:], mybir.ActivationFunctionType.Exp,
                scale=SCALE / (WS * WS))
            nc.vector.reduce_sum(
                dn[:], ex[:].rearrange("p (h g) -> p h g", g=G),
                axis=mybir.AxisListType.X,
            )
            nc.vector.reciprocal(rc[:], dn[:])
            # pf = OS * ex / dn  (o shipped as 16*o for the fp8 split)
            nc.vector.scalar_tensor_tensor(
                pf[:].rearrange("p (h g) -> p h g", g=G),
                ex[:].rearrange("p (h g) -> p h g", g=G),
                OS,
                rc[:].unsqueeze(2).broadcast_to((P, H, G)),
                op0=mybir.AluOpType.mult, op1=mybir.AluOpType.mult,
            )

            # o[t,(h,d)] = sum_g p[t,(h,g)] * v[t,(g,d)]  - batched per g on
            # the otherwise-idle GpSimd/Pool engine (4 mult + 3 accum ops of
            # [p, H*D], broadcast views; DVE keeps only the score dots)
            obf = obf_pool.tile([P, HID], BF16, name="obf", tag="obf")
            obf_sb[t] = obf
            tmp = wt_pool.tile([P, HID], BF16, tag="ta", name="ta")
            ob3 = obf[:].rearrange("p (h d) -> p h d", d=D)
            tm3 = tmp[:].rearrange("p (h d) -> p h d", d=D)
            pf3 = pf[:].rearrange("p (h g) -> p h g", g=G)
            vv = lambda g: vsb[:, g * D:(g + 1) * D].unsqueeze(1) \
                .broadcast_to((P, H, D))
            pp = lambda g: pf3[:, :, g:g + 1].broadcast_to((P, H, D))
            nc.gpsimd.tensor_tensor(ob3, vv(0), pp(0), op=mybir.AluOpType.mult)
            for g in range(1, G):
                nc.gpsimd.tensor_tensor(tm3, vv(g), pp(g),
                                        op=mybir.AluOpType.mult)
                nc.gpsimd.tensor_tensor(ob3, ob3, tm3,
                                        op=mybir.AluOpType.add)

        def transpose_split_o(t):
            # O^T via the DMA xbar (keeps the PE matmul-only):
            # ot[p, o*128+tok] = obf[tok, o*128+p]; then split to fp8 hi/lo.
            ot = ot_pool.tile([P, HID], BF16, name="ot", tag="ot")
            ot_sb[t] = ot
            nc.sync.dma_start_transpose(
                ot[:].rearrange("p (o t2) -> p o t2", t2=P), obf_sb[t][:])
            obf_sb[t] = None
            oth = oth_pool.tile([P, HID], F8, name="oth", tag="oth")
            otl = otl_pool.tile([P, HID], F8, name="otl", tag="otl")
            oth_sb[t] = oth
            otl_sb[t] = otl
            nc.scalar.copy(oth[:], ot[:])
            nc.gpsimd.tensor_sub(otl[:], ot[:], oth[:])
            ot_sb[t] = None

        def copy_qkv(which, c, ps, qsb, ksb, vsb):
            if which == "q":
                nc.scalar.copy(qsb[:, c * 512:(c + 1) * 512], ps[:])
            elif which == "k":
                nc.scalar.copy(ksb[:], ps[:])
            else:
                nc.scalar.mul(vsb[:], ps[:], 1.0 / WS)

        # ---- prologue: tiles 0+1 consume the qkv weight stream in arrival
        # order - six interleaved psum chains (3 from the qkv pool, 3
        # borrowed from the still-idle y pool) per half ----
        pro_sb = {}
        for tt in (0, 1):
            pro_sb[tt] = (
                qsb_pool.tile([P, HID], BF16, tag="q", name="q"),
                qsb_pool.tile([P, KV], BF16, tag="k", name="k"),
                qsb_pool.tile([P, KV], BF16, tag="v", name="v"),
            )
        for half in range(2):
            chains = (("q", 0), ("q", 1), ("k", 0)) if half == 0 else \
                     (("q", 2), ("q", 3), ("v", 0))
            pairs = []
            for tt in (0, 1):
                pool = qkv_ps_pool if tt == 0 else y_ps_pool
                tagname = "ps" if tt == 0 else "yps"
                for which, c in chains:
                    ps = pool.tile([P, 512], F32, name=tagname, tag=tagname)
                    pairs.append((ps, qkv_steps(tt, which, c), tt, which, c))
            emit_chain_pair([(ps, st) for ps, st, _, _, _ in pairs])
            for ps, _, tt, which, c in pairs:
                copy_qkv(which, c, ps, *pro_sb[tt])
        for tt in (0, 1):
            attn_middle(tt, *pro_sb[tt])
            transpose_split_o(tt)

        # ---- steady state: QKV(t) | Wo(t-4), software pipeline depth 4
        # (deep enough that the first y chains never wait on the woh/wol
        # weight stream) ----
        for t in range(2, NTT + 4):
            if 2 <= t <= 13:
                load_xh(t + 2)
                load_xl(t + 2)

            if t < NTT:
                qsb = qsb_pool.tile([P, HID], BF16, tag="q")
                ksb = qsb_pool.tile([P, KV], BF16, tag="k")
                vsb = qsb_pool.tile([P, KV], BF16, tag="v")

                # 3 pair-interleaved chains; paired chains share the
                # stationary x pair per step (back-to-back reuse)
                for pa, pb in ((("q", 0), ("k", 0)),
                               (("q", 1), ("q", 2)),
                               (("q", 3), ("v", 0))):
                    psa = qkv_ps_pool.tile([P, 512], F32, name="ps", tag="ps")
                    psb = qkv_ps_pool.tile([P, 512], F32, name="ps", tag="ps")
                    emit_chain_pair([
                        (psa, qkv_steps(t, pa[0], pa[1])),
                        (psb, qkv_steps(t, pb[0], pb[1])),
                    ])
                    copy_qkv(pa[0], pa[1], psa, qsb, ksb, vsb)
                    copy_qkv(pb[0], pb[1], psb, qsb, ksb, vsb)

                attn_middle(t, qsb, ksb, vsb)

            # Wo matmuls + y out for tile t-4
            if t - 4 >= 0:
                tw = t - 4
                if tw == NTT - 1:
                    # last tile: sequential chains, final ones narrow, so the
                    # post-matmul drain holds only one short copy+DMA
                    for c0, c1 in ((0, 512), (512, 1024), (1024, 1536),
                                   (1536, 1792), (1792, 2048)):
                        yps = y_ps_pool.tile([P, c1 - c0], F32,
                                             name="yps", tag="yps")
                        emit_chain_pair([(yps, y_steps(tw, c0, c1))])
                        ysb = ysb_pool.tile([P, c1 - c0], F32,
                                            name="ysb", tag="ysb")
                        nc.scalar.mul(ysb[:], yps[:], 1.0 / (WS * OS))
                        nc.sync.dma_start(
                            y[tw * P:(tw + 1) * P, c0:c1], ysb[:])
                    oth_sb[tw] = None
                    otl_sb[tw] = None
                    continue
                for sp in range(2):
                    ypa = y_ps_pool.tile([P, 512], F32, name="yps", tag="yps")
                    ypb = y_ps_pool.tile([P, 512], F32, name="yps", tag="yps")
                    emit_chain_pair([
                        (ypa, y_steps(tw, 2 * sp * 512, (2 * sp + 1) * 512)),
                        (ypb, y_steps(tw, (2 * sp + 1) * 512, (2 * sp + 2) * 512)),
                    ])
                    for yps, s_ in ((ypa, 2 * sp), (ypb, 2 * sp + 1)):
                        ysb = ysb_pool.tile([P, 512], F32, name="ysb", tag="ysb")
                        nc.scalar.mul(ysb[:], yps[:], 1.0 / (WS * OS))
                        nc.sync.dma_start(
                            y[tw * P:(tw + 1) * P, s_ * 512:(s_ + 1) * 512],
                            ysb[:])
                oth_sb[tw] = None
                otl_sb[tw] = None

            if 2 <= t < NTT:
                transpose_split_o(t)

    nc.compile()
    return nc


def _build_bias(has_bias: bool = True) -> bass.Bass:
    """Original (slower) path, kept for the biased case."""
    nc = bacc.Bacc("TRN2")
    x = nc.dram_tensor("x", [TPC, HID], BF16, kind="ExternalInput")
    wq = nc.dram_tensor("wq", [HID, HID], BF16, kind="ExternalInput")
    wk = nc.dram_tensor("wk", [HID, KV], BF16, kind="ExternalInput")
    wv = nc.dram_tensor("wv", [HID, KV], BF16, kind="ExternalInput")
    wo = nc.dram_tensor("wo", [HID, HID], BF16, kind="ExternalInput")
    if has_bias:
        bqkv = nc.dram_tensor("bqkv", [1, HID + 2 * KV], F32, kind="ExternalInput")
        bo = nc.dram_tensor("bo", [1, HID], F32, kind="ExternalInput")
    y = nc.dram_tensor("y", [TPC, HID], F32, kind="ExternalOutput")

    with tile.TileContext(nc) as tc, ExitStack() as ctx:
        const_pool = ctx.enter_context(tc.tile_pool(name="const", bufs=1))
        ident = const_pool.tile([P, P], BF16)
        make_identity(nc, ident[:])

        if has_bias:
            bias_qkv = const_pool.tile([P, HID + 2 * KV], F32)
            nc.sync.dma_start(bias_qkv[:], bqkv[0:1, :].broadcast_to((P, HID + 2 * KV)))
            bias_o = const_pool.tile([P, HID], F32)
            nc.sync.dma_start(bias_o[:], bo[0:1, :].broadcast_to((P, HID)))

        # O^T staging for the whole core: [o_block(16) x tokens(2048)] bf16
        ofm_pool = ctx.enter_context(tc.tile_pool(name="ofm", bufs=1))
        ofm = ofm_pool.tile([P, NI * TPC], BF16)

        kv_pool = ctx.enter_context(tc.tile_pool(name="wkv", bufs=1))
        wk_sb = []
        wv_sb = []
        for i in range(NI):
            wk_t = kv_pool.tile([P, KV], BF16, tag=f"wk{i}")
            nc.sync.dma_start(wk_t[:], wk[i * P:(i + 1) * P, :])
            wk_sb.append(wk_t)
            wv_t = kv_pool.tile([P, KV], BF16, tag=f"wv{i}")
            nc.sync.dma_start(wv_t[:], wv[i * P:(i + 1) * P, :])
            wv_sb.append(wv_t)

        pt_pool = ctx.enter_context(tc.tile_pool(name="pt", bufs=2, space="PSUM"))
        mm_pool = ctx.enter_context(tc.tile_pool(name="mm", bufs=3, space="PSUM"))

        # ---------------- Phase A: QKV projections + attention ----------------
        with tc.tile_pool(name="wqp", bufs=1) as wq_pool, \
             tc.tile_pool(name="xt", bufs=3) as xt_pool, \
             tc.tile_pool(name="xfm", bufs=1) as xfm_pool, \
             tc.tile_pool(name="qkv", bufs=1) as qkv_pool, \
             tc.tile_pool(name="attn", bufs=2) as attn_pool, \
             tc.tile_pool(name="oacc", bufs=1) as oacc_pool, \
             tc.tile_pool(name="obf", bufs=1) as obf_pool:
            wq_sb = []
            for i in range(NI):
                wq_t = wq_pool.tile([P, HID], BF16, tag=f"wq{i}")
                nc.sync.dma_start(wq_t[:], wq[i * P:(i + 1) * P, :])
                wq_sb.append(wq_t)

            for t in range(NTT):
                xt = xt_pool.tile([P, HID], BF16)
                nc.sync.dma_start(xt[:], x[t * P:(t + 1) * P, :])

                # transpose X tile to feature-major [i, t] (16 blocks of 128x128)
                xfm = xfm_pool.tile([P, HID], BF16)
                for j in range(4):
                    pt = pt_pool.tile([P, 512], BF16)
                    for k in range(4):
                        blk = 4 * j + k
                        nc.tensor.transpose(
                            pt[:, k * P:(k + 1) * P],
                            xt[:, blk * P:(blk + 1) * P],
                            ident[:],
                        )
                    nc.vector.tensor_copy(xfm[:, j * 512:(j + 1) * 512], pt[:])

                # QKV projections, token-major out: [t(128part), 3072]
                qkv = qkv_pool.tile([P, HID + 2 * KV], F32)
                for s in range(6):
                    ps = mm_pool.tile([P, 512], F32)
                    for i in range(NI):
                        if s < 4:
                            rhs = wq_sb[i][:, s * 512:(s + 1) * 512]
                        elif s == 4:
                            rhs = wk_sb[i][:]
                        else:
                            rhs = wv_sb[i][:]
                        nc.tensor.matmul(
                            ps[:], xfm[:, i * P:(i + 1) * P], rhs,
                            start=(i == 0), stop=(i == NI - 1),
                        )
                    if has_bias:
                        nc.vector.tensor_add(
                            qkv[:, s * 512:(s + 1) * 512], ps[:],
                            bias_qkv[:, s * 512:(s + 1) * 512],
                        )
                    else:
                        nc.vector.tensor_copy(qkv[:, s * 512:(s + 1) * 512], ps[:])

                # scores[t, h, g] = <q_h, k_g> * SCALE   (fused mult+reduce)
                sc = attn_pool.tile([P, H * G], F32, tag="sc")
                junk = attn_pool.tile([P, D], F32, tag="junk")
                for h in range(H):
                    for g in range(G):
                        nc.vector.scalar_tensor_tensor(
                            junk[:],
                            qkv[:, h * D:(h + 1) * D],
                            SCALE,
                            qkv[:, HID + g * D:HID + (g + 1) * D],
                            op0=mybir.AluOpType.mult,
                            op1=mybir.AluOpType.mult,
                            accum_out=sc[:, ds(h * G + g, 1)],
                        )

                # softmax over g (4); denominator folded into final scale
                ex = attn_pool.tile([P, H * G], F32, tag="ex")
                nc.scalar.activation(ex[:], sc[:], mybir.ActivationFunctionType.Exp)
                dn = attn_pool.tile([P, H], F32, tag="dn")
                nc.vector.reduce_sum(
                    dn[:], ex[:].rearrange("p (h g) -> p h g", g=G),
                    axis=mybir.AxisListType.X,
                )
                rc = attn_pool.tile([P, H], F32, tag="rc")
                nc.vector.reciprocal(rc[:], dn[:])

                # o[t, h*D+d] = (sum_g ex[t,h,g] * v[t, g*D+d]) * rc[t,h]
                acc = oacc_pool.tile([P, HID], F32, tag="acc")
                tmp = oacc_pool.tile([P, HID], F32, tag="tmp")
                obf = obf_pool.tile([P, HID], BF16)
                ab = [acc, tmp]
                for h in range(H):
                    hs = ds(h * D, D)
                    nc.vector.tensor_scalar_mul(
                        ab[0][:, hs],
                        qkv[:, HID + KV:HID + KV + D],
                        ex[:, ds(h * G, 1)],
                    )
                    for g in range(1, G):
                        nc.vector.scalar_tensor_tensor(
                            ab[g % 2][:, hs],
                            qkv[:, HID + KV + g * D:HID + KV + (g + 1) * D],
                            ex[:, ds(h * G + g, 1)],
                            ab[(g - 1) % 2][:, hs],
                            op0=mybir.AluOpType.mult,
                            op1=mybir.AluOpType.add,
                        )
                    nc.vector.tensor_scalar_mul(
                        obf[:, hs], ab[(G - 1) % 2][:, hs], rc[:, ds(h, 1)])

                # transpose O tile into ofm [o_block, token]
                for j in range(4):
                    pt = pt_pool.tile([P, 512], BF16)
                    for k in range(4):
                        blk = 4 * j + k
                        nc.tensor.transpose(
                            pt[:, k * P:(k + 1) * P],
                            obf[:, blk * P:(blk + 1) * P],
                            ident[:],
                        )
                    nc.vector.tensor_copy(
                        ofm[:].rearrange("p (o t) -> p o t", t=TPC)
                              [:, 4 * j:4 * j + 4, t * P:(t + 1) * P],
                        pt[:].rearrange("p (o t) -> p o t", t=P),
                    )

        # ---------------- Phase B: output projection ----------------
        with tc.tile_pool(name="wop", bufs=1) as wo_pool, \
             tc.tile_pool(name="yt", bufs=3) as yt_pool:
            wo_sb = []
            for i in range(NI):
                wo_t = wo_pool.tile([P, HID], BF16, tag=f"wo{i}")
                nc.sync.dma_start(wo_t[:], wo[i * P:(i + 1) * P, :])
                wo_sb.append(wo_t)

            for t in range(NTT):
                for s in range(4):
                    ps = mm_pool.tile([P, 512], F32)
                    for o in range(NI):
                        nc.tensor.matmul(
                            ps[:],
                            ofm[:, ds(o * TPC + t * P, P)],
                            wo_sb[o][:, s * 512:(s + 1) * 512],
                            start=(o == 0), stop=(o == NI - 1),
                        )
                    yt = yt_pool.tile([P, 512], F32)
                    if has_bias:
                        nc.vector.tensor_add(
                            yt[:], ps[:], bias_o[:, s * 512:(s + 1) * 512])
                    else:
                        nc.vector.tensor_copy(yt[:], ps[:])
                    nc.sync.dma_start(
                        y[t * P:(t + 1) * P, s * 512:(s + 1) * 512], yt[:])

    nc.compile()
    return nc


def _build(has_bias: bool) -> bass.Bass:
    return _build_bias(True) if has_bias else _build_fp8()


def kernel(hidden_states, Wq, bq, Wk, bk, Wv, bv, Wo, bo, _profile=None):
    has_bias = bool(np.any(bq) or np.any(bk) or np.any(bv) or np.any(bo))
    key = has_bias
    if key not in _cache:
        _cache[key] = _build(has_bias)
    nc = _cache[key]

    x_flat = np.ascontiguousarray(
        np.asarray(hidden_states, dtype=np.float32).reshape(NTOK, HID))

    in_maps = []
    if has_bias:
        bf = ml_dtypes.bfloat16
        xb = x_flat.astype(bf)
        wq_b = np.asarray(Wq, dtype=np.float32).astype(bf)
        wk_b = np.asarray(Wk, dtype=np.float32).astype(bf)
        wv_b = np.asarray(Wv, dtype=np.float32).astype(bf)
        wo_b = np.asarray(Wo, dtype=np.float32).astype(bf)
        for c in range(NCORES):
            m = {
                "x": np.ascontiguousarray(xb[c * TPC:(c + 1) * TPC]),
                "wq": wq_b, "wk": wk_b, "wv": wv_b, "wo": wo_b,
                "bqkv": np.concatenate([
                    np.asarray(bq, np.float32), np.asarray(bk, np.float32),
                    np.asarray(bv, np.float32)]).reshape(1, HID + 2 * KV),
                "bo": np.asarray(bo, np.float32).reshape(1, HID),
            }
            in_maps.append(m)
    else:
        e4 = ml_dtypes.float8_e4m3
        xh8 = x_flat.astype(e4)
        xl8 = (x_flat - xh8.astype(np.float32)).astype(e4)

        def wsplit(W):
            Wf = np.asarray(W, dtype=np.float32) * WS
            hi = Wf.astype(e4)
            lo = (Wf - hi.astype(np.float32)).astype(e4)
            return np.ascontiguousarray(hi), np.ascontiguousarray(lo)

        wq8 = np.ascontiguousarray(
            (np.asarray(Wq, np.float32) * WS).astype(e4))
        wk8 = np.ascontiguousarray(
            (np.asarray(Wk, np.float32) * WS).astype(e4))
        wvh8, wvl8 = wsplit(Wv)
        woh8, wol8 = wsplit(Wo)

        def pret(a):
            # host pre-transpose: row (t*128+p), col (i*128+tok) <- x[(t,tok),(i,p)]
            return np.ascontiguousarray(
                a.reshape(NTT, P, NI, P).transpose(0, 3, 2, 1).reshape(TPC, HID))

        for c in range(NCORES):
            m = {
                "xh": pret(xh8[c * TPC:(c + 1) * TPC]),
                "xl": pret(xl8[c * TPC:(c + 1) * TPC]),
                "wq": wq8, "wk": wk8,
                "wvh": wvh8, "wvl": wvl8,
                "woh": woh8, "wol": wol8,
            }
            in_maps.append(m)

    kwargs = dict(_profile) if _profile else {}
    kwargs.pop("result", None)
    res = run_bass_kernel_spmd(nc, in_maps, list(range(NCORES)), **kwargs)
    out = np.concatenate([r["y"] for r in res.results], axis=0)
    if _profile is not None:
        _profile["result"] = res
    return out.reshape(B, S, HID).astype(np.float32)
